# revision 16
# baseline (speedup 1.0000x reference)
"""Mixtral decoder layer (attention + top-2 MoE) on 8 TRN2 NeuronCores.

Sharding: token-parallel attention (512 tokens/core, sliding-window KV
overlap comes in via per-core window inputs), expert-parallel sparse MoE
(1 expert/core, token dispatch via on-device cumsum + indirect DMA
gather/scatter), AllGather of post-attention hidden states + router
logits, ReduceScatter of expert outputs.

Self-contained: hardcodes all shapes; host preprocessing only slices /
casts / builds constant tables.
"""
import os
import numpy as np
import ml_dtypes

import concourse.bass as bass
from concourse import bacc
import concourse.mybir as mybir
import concourse.tile as tile
from concourse.bass_utils import run_bass_kernel_spmd

B, S, H = 2, 2048, 1024
NQ, NKV, HD = 16, 4, 64
I, E, TOP_K = 3584, 8, 2
WIN = 512
EPS = 1e-5
CORES = 8
TPC = 512            # tokens per core
T = B * S            # 4096
TC = T // 128        # 32 token columns for gating math
C = 1152             # expert capacity (9 * 128)
CF = C // 128        # 9 slot tiles
TB = 384             # FFN token-block
NB = C // TB         # 3 blocks
IT = I // 128        # 28 i-tiles
TRASH = C
TMAPN = 1280
NEG = -30000.0

F32, F16, I32 = mybir.dt.float32, mybir.dt.float16, mybir.dt.int32
AOP = mybir.AluOpType
AF = mybir.ActivationFunctionType

_DEBUG = bool(int(os.environ.get("BASS_MOE_DEBUG", "0")))
_nc_cache = None


def build_kernel():
    nc = bacc.Bacc("TRN2", target_bir_lowering=False)
    RG = [list(range(CORES))]

    # ---------------- parameters ----------------
    xw16 = nc.declare_dram_parameter("xw16", [1024, H], F16, isOutput=False)
    xo32 = nc.declare_dram_parameter("xo32", [TPC, H], F32, isOutput=False)
    wq = nc.declare_dram_parameter("wq", [H, NQ * HD], F16, isOutput=False)
    wk = nc.declare_dram_parameter("wk", [H, 2 * NKV * HD], F16, isOutput=False)
    wv = nc.declare_dram_parameter("wv", [H, NKV * HD], F16, isOutput=False)
    wo = nc.declare_dram_parameter("wo", [NQ * HD, H], F16, isOutput=False)
    wr = nc.declare_dram_parameter("wr", [H, E], F32, isOutput=False)
    wg = nc.declare_dram_parameter("wg", [H, I], F16, isOutput=False)
    wu = nc.declare_dram_parameter("wu", [H, I], F16, isOutput=False)
    wd = nc.declare_dram_parameter("wd", [I, H], F16, isOutput=False)
    cosw = nc.declare_dram_parameter("cosw", [128, 1024], F32, isOutput=False)
    sinw = nc.declare_dram_parameter("sinw", [128, 1024], F32, isOutput=False)
    m4 = nc.declare_dram_parameter("m4", [4, 128, 640], F16, isOutput=False)
    rotT = nc.declare_dram_parameter("rotT", [128, 128], F32, isOutput=False)
    id16 = nc.declare_dram_parameter("id16", [128, 128], F16, isOutput=False)
    id32 = nc.declare_dram_parameter("id32", [128, 128], F32, isOutput=False)
    u128 = nc.declare_dram_parameter("u128", [128, 128], F16, isOutput=False)
    sl32 = nc.declare_dram_parameter("sl32", [32, 32], F32, isOutput=False)
    ones_l = nc.declare_dram_parameter("ones_l", [32, 128], F16, isOutput=False)
    iota_w = nc.declare_dram_parameter("iota_w", [128, 3 * TC], F32, isOutput=False)
    esel = nc.declare_dram_parameter("esel", [128, E], F32, isOutput=False)

    out_dec = nc.declare_dram_parameter("out_dec", [TPC, H], F32, isOutput=True)
    out_rl = nc.declare_dram_parameter("out_rl", [TPC, E], F32, isOutput=True)
    if _DEBUG:
        dbg_x2 = nc.declare_dram_parameter("dbg_x2", [TPC, H], F32, isOutput=True)
        dbg_moe = nc.declare_dram_parameter("dbg_moe", [TPC, H], F32, isOutput=True)

    # ---------------- internal DRAM ----------------
    ag_ins = [nc.dram_tensor(f"ag_in{k}", [128, H], F16) for k in range(4)]
    ag_hs = nc.dram_tensor("ag_hs", [T + 128, H], F16, addr_space="Shared")
    lg_in = nc.dram_tensor("lg_in", [TPC, E], F32)
    ag_lg = nc.dram_tensor("ag_lg", [T, E], F32, addr_space="Shared")
    tmds = [nc.dram_tensor(f"tmap_dram{k}", [TMAPN, 3], F32) for k in range(16)]
    accum = nc.dram_tensor("accum", [T + 128, H], F16)
    rs_out = nc.dram_tensor("rs_out", [TPC, H], F16)

    with tile.TileContext(nc) as tc:
        # persistent pool (consts + tensors alive across phases)
        pp = tc.alloc_tile_pool(name="persist", bufs=1)
        c_id16 = pp.tile([128, 128], F16)
        c_id32 = pp.tile([128, 128], F32)
        c_u128 = pp.tile([128, 128], F16)
        c_sl32 = pp.tile([32, 32], F32)
        c_onesl = pp.tile([32, 128], F16)
        c_rotT = pp.tile([128, 128], F32)
        c_esel = pp.tile([128, E], F32)
        tw = pp.tile([128, 3 * TC], F32)
        nc.sync.dma_start(out=c_id16[:], in_=id16[:, :])
        nc.sync.dma_start(out=c_id32[:], in_=id32[:, :])
        nc.sync.dma_start(out=c_u128[:], in_=u128[:, :])
        nc.sync.dma_start(out=c_sl32[:], in_=sl32[:, :])
        nc.sync.dma_start(out=c_onesl[:], in_=ones_l[:, :])
        nc.sync.dma_start(out=c_rotT[:], in_=rotT[:, :])
        nc.sync.dma_start(out=c_esel[:], in_=esel[:, :])
        nc.sync.dma_start(out=tw[:], in_=iota_w[:, :])
        x2 = pp.tile([128, 4, H], F32)          # post-attention + residual (own tokens)
        logit_sb = pp.tile([128, 4, E], F32)    # own-slice router logits

        # =========== Phase A/B/C: attention ===========
        ap = tc.alloc_tile_pool(name="attn", bufs=1)
        ap1 = tc.alloc_tile_pool(name="attn1", bufs=1)
        ap2 = tc.alloc_tile_pool(name="attn2", bufs=2)
        aps = tc.alloc_tile_pool(name="attn_ps", bufs=1, space="PSUM")
        aps2 = tc.alloc_tile_pool(name="attn_ps2", bufs=2, space="PSUM")

        c_cos = ap.tile([128, 1024], F32)
        c_sin = ap.tile([128, 1024], F32)
        c_m4 = ap.tile([128, 4, 640], F16)
        nc.sync.dma_start(out=c_cos[:], in_=cosw[:, :])
        nc.sync.dma_start(out=c_sin[:], in_=sinw[:, :])
        nc.sync.dma_start(out=c_m4[:], in_=m4.ap().rearrange("q p k -> p q k"))
        wq_sb = ap.tile([128, 8, NQ * HD], F16)
        wk_sb = ap.tile([128, 8, 2 * NKV * HD], F16)
        wv_sb = ap.tile([128, 8, NKV * HD], F16)
        wo_sb = ap.tile([128, 8, H], F16)
        nc.sync.dma_start(out=wq_sb[:], in_=wq.ap().rearrange("(hc p) n -> p hc n", p=128))
        nc.sync.dma_start(out=wk_sb[:], in_=wk.ap().rearrange("(hc p) n -> p hc n", p=128))
        nc.sync.dma_start(out=wv_sb[:], in_=wv.ap().rearrange("(hc p) n -> p hc n", p=128))
        nc.sync.dma_start(out=wo_sb[:], in_=wo.ap().rearrange("(nc p) h -> p nc h", p=128))

        # A: rmsnorm(x_win) -> hs (token-major), then transpose -> hsT
        hsT = ap.tile([128, 8, 1024], F16)      # [h%128, h//128, window tok]
        for tt in range(8):
            xt = ap2.tile([128, H], F16, tag="xw")
            nc.sync.dma_start(out=xt[:], in_=xw16[128 * tt:128 * (tt + 1), :])
            sq = ap1.tile([128, H], F32, tag="sq")
            nc.vector.tensor_tensor(out=sq[:], in0=xt[:], in1=xt[:], op=AOP.mult)
            ssum = ap2.tile([128, 1], F32, tag="ssum")
            nc.vector.tensor_reduce(out=ssum[:], in_=sq[:], axis=mybir.AxisListType.X, op=AOP.add)
            nc.vector.tensor_scalar(out=ssum[:], in0=ssum[:], scalar1=1.0 / H, op0=AOP.mult,
                                    scalar2=EPS, op1=AOP.add)
            nc.vector.reciprocal(out=ssum[:], in_=ssum[:])
            rs_ = ap2.tile([128, 1], F32, tag="rs")
            nc.scalar.activation(rs_[:], ssum[:], AF.Sqrt)
            hst = ap2.tile([128, H], F16, tag="hst")
            nc.vector.tensor_scalar_mul(hst[:], xt[:], rs_[:, :1])
            for hc in range(8):
                tp = aps2.tile([128, 128], F16, space="PSUM", tag="tp")
                nc.tensor.transpose(out=tp[:], in_=hst[:, 128 * hc:128 * (hc + 1)], identity=c_id16[:])
                nc.scalar.copy(out=hsT[:, hc, 128 * tt:128 * (tt + 1)], in_=tp[:])

        # B: QKV projections + rope
        qroT = ap.tile([128, 8, 512], F16)      # [(2 heads)*64, m, q tok]
        kroT = ap.tile([128, 4, 1024], F16)
        v16 = ap.tile([128, 8, NKV * HD], F16)  # token-major V
        for m in range(8):
            pq = aps.tile([128, 512], F32, space="PSUM", tag="med")
            for hc in range(8):
                nc.tensor.matmul(pq[:], wq_sb[:, hc, 128 * m:128 * (m + 1)],
                                 hsT[:, hc, 512:1024], start=(hc == 0), stop=(hc == 7))
            qsb = ap2.tile([128, 512], F32, tag="qsb")
            nc.vector.tensor_copy(qsb[:], pq[:])
            prot = aps.tile([128, 512], F32, space="PSUM", tag="med")
            nc.tensor.matmul(prot[:], c_rotT[:], qsb[:], start=True, stop=True)
            t1 = ap2.tile([128, 512], F32, tag="t1")
            nc.vector.tensor_tensor(out=t1[:], in0=qsb[:], in1=c_cos[:, 512:1024], op=AOP.mult)
            t2 = ap2.tile([128, 512], F32, tag="t2")
            nc.vector.tensor_copy(t2[:], prot[:])
            nc.vector.tensor_tensor(out=t2[:], in0=t2[:], in1=c_sin[:, 512:1024], op=AOP.mult)
            nc.vector.tensor_tensor(out=qroT[:, m, :], in0=t1[:], in1=t2[:], op=AOP.add)
        for m in range(4):
            pk = aps.tile([128, 1024], F32, space="PSUM", tag="big")
            for half in range(2):
                for hc in range(8):
                    nc.tensor.matmul(pk[:, 512 * half:512 * (half + 1)],
                                     wk_sb[:, hc, 128 * m:128 * (m + 1)],
                                     hsT[:, hc, 512 * half:512 * (half + 1)],
                                     start=(hc == 0), stop=(hc == 7))
            ksb = ap1.tile([128, 1024], F32, tag="ksb")
            nc.vector.tensor_copy(ksb[:], pk[:])
            prot = aps.tile([128, 1024], F32, space="PSUM", tag="big")
            nc.tensor.matmul(prot[:, 0:512], c_rotT[:], ksb[:, 0:512], start=True, stop=True)
            nc.tensor.matmul(prot[:, 512:1024], c_rotT[:], ksb[:, 512:1024], start=True, stop=True)
            t1 = ap1.tile([128, 1024], F32, tag="t1k")
            nc.vector.tensor_tensor(out=t1[:], in0=ksb[:], in1=c_cos[:], op=AOP.mult)
            t2 = ap1.tile([128, 1024], F32, tag="t2k")
            nc.vector.tensor_copy(t2[:], prot[:])
            nc.vector.tensor_tensor(out=t2[:], in0=t2[:], in1=c_sin[:], op=AOP.mult)
            nc.vector.tensor_tensor(out=kroT[:, m, :], in0=t1[:], in1=t2[:], op=AOP.add)
        for tt in range(8):
            pv = aps.tile([128, NKV * HD], F32, space="PSUM", tag="med")
            for hc in range(8):
                nc.tensor.matmul(pv[:], hsT[:, hc, 128 * tt:128 * (tt + 1)],
                                 wv_sb[:, hc, :], start=(hc == 0), stop=(hc == 7))
            nc.vector.tensor_copy(v16[:, tt, :], pv[:])

        # C: per q-tile attention
        wr_sb = ap.tile([128, 8, E], F32)
        nc.sync.dma_start(out=wr_sb[:], in_=wr.ap().rearrange("(hc p) e -> p hc e", p=128))
        for qt in range(4):
            acat = ap.tile([128, 8, 128], F16, tag="acat")
            for kh in range(NKV):
                for ql in range(4):
                    qh = 4 * kh + ql
                    ps = aps.tile([128, 640], F32, space="PSUM", tag="score")
                    par = 64 * (qh % 2)
                    lq = qroT[par:par + 64, qh // 2, 128 * qt:128 * (qt + 1)]
                    rk = kroT[par:par + 64, kh, :]
                    nc.tensor.matmul(ps[:, 0:512], lq, rk[:, 128 * qt:128 * qt + 512],
                                     start=True, stop=True)
                    nc.tensor.matmul(ps[:, 512:640], lq, rk[:, 128 * qt + 512:128 * qt + 640],
                                     start=True, stop=True)
                    nc.vector.tensor_tensor(out=ps[:], in0=ps[:], in1=c_m4[:, qt, :], op=AOP.add)
                    mx = ap2.tile([128, 1], F32, tag="mx")
                    nc.vector.tensor_reduce(out=mx[:], in_=ps[:], axis=mybir.AxisListType.X, op=AOP.max)
                    nc.vector.tensor_scalar_mul(mx[:], mx[:], -1.0)
                    pr = ap2.tile([128, 640], F16, tag="pr")
                    nc.scalar.activation(pr[:], ps[:], AF.Exp, bias=mx[:, :1])
                    se = ap2.tile([128, 1], F32, tag="se")
                    nc.vector.tensor_reduce(out=se[:], in_=pr[:], axis=mybir.AxisListType.X, op=AOP.add)
                    nc.vector.reciprocal(out=se[:], in_=se[:])
                    nc.vector.tensor_scalar_mul(pr[:], pr[:], se[:, :1])
                    prT = ap2.tile([128, 5, 128], F16, tag="prT")
                    for kc in range(5):
                        tp = aps2.tile([128, 128], F16, space="PSUM", tag="tp")
                        nc.tensor.transpose(out=tp[:], in_=pr[:, 128 * kc:128 * (kc + 1)],
                                            identity=c_id16[:])
                        nc.scalar.copy(out=prT[:, kc, :], in_=tp[:])
                    pa = aps.tile([64, 128], F32, space="PSUM", tag="med")
                    for kc in range(5):
                        nc.tensor.matmul(pa[:], v16[:, qt + kc, 64 * kh:64 * kh + 64],
                                         prT[:, kc, :], start=(kc == 0), stop=(kc == 4))
                    patmp = ap2.tile([64, 128], F16, tag="patmp")
                    nc.scalar.copy(out=patmp[:], in_=pa[:])
                    nc.sync.dma_start(out=acat[64 * (qh % 2):64 * (qh % 2) + 64, qh // 2, :],
                                      in_=patmp[:])
            po = aps.tile([128, 1024], F32, space="PSUM", tag="big")
            for half in range(2):
                for b in range(8):
                    nc.tensor.matmul(po[:, 512 * half:512 * (half + 1)], acat[:, b, :],
                                     wo_sb[:, b, 512 * half:512 * (half + 1)],
                                     start=(b == 0), stop=(b == 7))
            xo_t = ap1.tile([128, H], F32, tag="xo")
            nc.sync.dma_start(out=xo_t[:], in_=xo32[128 * qt:128 * (qt + 1), :])
            nc.vector.tensor_tensor(out=x2[:, qt, :], in0=po[:], in1=xo_t[:], op=AOP.add)

            # norm2 + hs2 (fp16, to AG) + hs2T (f32, router)
            sq = ap1.tile([128, H], F32, tag="sq2")
            nc.vector.tensor_tensor(out=sq[:], in0=x2[:, qt, :], in1=x2[:, qt, :], op=AOP.mult)
            ssum = ap2.tile([128, 1], F32, tag="ssum2")
            nc.vector.tensor_reduce(out=ssum[:], in_=sq[:], axis=mybir.AxisListType.X, op=AOP.add)
            nc.vector.tensor_scalar(out=ssum[:], in0=ssum[:], scalar1=1.0 / H, op0=AOP.mult,
                                    scalar2=EPS, op1=AOP.add)
            nc.vector.reciprocal(out=ssum[:], in_=ssum[:])
            rs_ = ap2.tile([128, 1], F32, tag="rs2")
            nc.scalar.activation(rs_[:], ssum[:], AF.Sqrt)
            hs2_32 = ap1.tile([128, H], F32, tag="hs2_32")
            nc.vector.tensor_scalar_mul(hs2_32[:], x2[:, qt, :], rs_[:, :1])
            hs2_16 = ap2.tile([128, H], F16, tag="hs2_16")
            nc.vector.tensor_copy(hs2_16[:], hs2_32[:])
            nc.sync.dma_start(out=ag_ins[qt][:, :], in_=hs2_16[:])
            nc.gpsimd.collective_compute("AllGather", AOP.bypass, replica_groups=RG,
                                         ins=[ag_ins[qt][:, :]],
                                         outs=[ag_hs[1024 * qt:1024 * (qt + 1), :]])
            # router: logits = hs2 @ wr  (f32; lhsT = hs2T chunks)
            plg = aps.tile([128, E], F32, space="PSUM", tag="med")
            for hc in range(8):
                tp32 = aps2.tile([128, 128], F32, space="PSUM", tag="tp")
                nc.tensor.transpose(out=tp32[:], in_=hs2_32[:, 128 * hc:128 * (hc + 1)],
                                    identity=c_id32[:])
                h2T = ap2.tile([128, 128], F32, tag="h2T")
                nc.vector.tensor_copy(h2T[:], tp32[:])
                nc.tensor.matmul(plg[:], h2T[:], wr_sb[:, hc, :], start=(hc == 0), stop=(hc == 7))
            nc.vector.tensor_copy(logit_sb[:, qt, :], plg[:])
            nc.sync.dma_start(out=lg_in[128 * qt:128 * (qt + 1), :], in_=logit_sb[:, qt, :])
            nc.sync.dma_start(out=out_rl[128 * qt:128 * (qt + 1), :], in_=logit_sb[:, qt, :])
            if _DEBUG:
                nc.sync.dma_start(out=dbg_x2[128 * qt:128 * (qt + 1), :], in_=x2[:, qt, :])

        for pool in (aps2, aps, ap2, ap1, ap):
            pool.release()

        # =========== AG collectives ===========
        cp = tc.alloc_tile_pool(name="coll", bufs=1)
        zt = cp.tile([128, H], F16)
        nc.vector.memset(zt[:], 0.0)
        nc.sync.dma_start(out=ag_hs[T:T + 128, :], in_=zt[:])
        nc.gpsimd.collective_compute("AllGather", AOP.bypass, replica_groups=RG,
                                     ins=[lg_in[:, :]], outs=[ag_lg[:, :]])

        # =========== Phase E: gating over all T tokens ===========
        gp = tc.alloc_tile_pool(name="gate", bufs=1)
        gp2 = tc.alloc_tile_pool(name="gate2", bufs=3)
        gps = tc.alloc_tile_pool(name="gate_ps", bufs=2, space="PSUM")
        mask32 = gp.tile([128, TC], F32)
        lt_all = gp.tile([128, TC, E], F32)
        nc.sync.dma_start(out=lt_all[:], in_=ag_lg.ap().rearrange("(c p) e -> p c e", p=128))
        m1 = gp.tile([128, TC], F32)
        nc.vector.tensor_reduce(out=m1[:], in_=lt_all[:], axis=mybir.AxisListType.X, op=AOP.max)
        m1b = m1[:].to_broadcast([128, TC, E])
        iseq = gp.tile([128, TC, E], F32)
        nc.vector.tensor_tensor(out=iseq[:], in0=lt_all[:], in1=m1b, op=AOP.is_equal)
        lt2 = gp.tile([128, TC, E], F32)
        nc.vector.scalar_tensor_tensor(out=lt2[:], in0=iseq[:], scalar=-1e9, in1=lt_all[:],
                                       op0=AOP.mult, op1=AOP.add)
        m2 = gp.tile([128, TC], F32)
        nc.vector.tensor_reduce(out=m2[:], in_=lt2[:], axis=mybir.AxisListType.X, op=AOP.max)
        eselb = c_esel[:].rearrange("p e -> p () e").to_broadcast([128, TC, E])
        lesel = gp.tile([128, TC, E], F32)
        nc.vector.tensor_tensor(out=lesel[:], in0=lt_all[:], in1=eselb, op=AOP.mult)
        le = gp.tile([128, TC], F32)
        nc.vector.tensor_reduce(out=le[:], in_=lesel[:], axis=mybir.AxisListType.X, op=AOP.add)
        nc.vector.tensor_tensor(out=mask32[:], in0=le[:], in1=m2[:], op=AOP.is_ge)
        d21 = gp.tile([128, TC], F32)
        nc.vector.tensor_tensor(out=d21[:], in0=m2[:], in1=m1[:], op=AOP.subtract)
        e2 = gp.tile([128, TC], F32)
        nc.scalar.activation(e2[:], d21[:], AF.Exp)
        nc.vector.tensor_scalar_add(e2[:], e2[:], 1.0)
        nc.vector.reciprocal(out=e2[:], in_=e2[:])
        dle = gp.tile([128, TC], F32)
        nc.vector.tensor_tensor(out=dle[:], in0=le[:], in1=m1[:], op=AOP.subtract)
        ew = gp.tile([128, TC], F32)
        nc.scalar.activation(ew[:], dle[:], AF.Exp)
        nc.vector.tensor_tensor(out=ew[:], in0=ew[:], in1=e2[:], op=AOP.mult)
        nc.vector.tensor_tensor(out=ew[:], in0=ew[:], in1=mask32[:], op=AOP.mult)
        nc.vector.tensor_copy(tw[:, 2:3 * TC:3], ew[:])

        # =========== Phase F: dispatch ===========
        mask16 = gp.tile([128, TC], F16)
        nc.vector.tensor_copy(mask16[:], mask32[:])
        colcum = gps.tile([128, TC], F32, space="PSUM", tag="colcum")
        nc.tensor.matmul(colcum[:], c_u128[:], mask16[:], start=True, stop=True)
        ones128 = gp.tile([128, 1], F16)
        nc.vector.memset(ones128[:], 1.0)
        counts_p = gps.tile([32, 1], F32, space="PSUM", tag="counts")
        nc.tensor.matmul(counts_p[:], mask16[:], ones128[:], start=True, stop=True)
        counts = gp.tile([32, 1], F32)
        nc.vector.tensor_copy(counts[:], counts_p[:])
        slc = gp.tile([32, 32], F16)
        nc.vector.tensor_scalar_mul(slc[:], c_sl32[:], counts[:, :1])
        carry = gps.tile([128, TC], F32, space="PSUM", tag="carry")
        nc.tensor.matmul(carry[:], c_onesl[:], slc[:, :TC], start=True, stop=True)
        pos = gp.tile([128, TC], F32)
        nc.vector.tensor_copy(pos[:], colcum[:])
        nc.vector.tensor_tensor(out=pos[:], in0=pos[:], in1=carry[:], op=AOP.add)
        nc.vector.tensor_tensor(out=pos[:], in0=pos[:], in1=mask32[:], op=AOP.subtract)
        nc.vector.tensor_scalar_add(pos[:], pos[:], -float(TRASH))
        nc.vector.tensor_tensor(out=pos[:], in0=pos[:], in1=mask32[:], op=AOP.mult)
        nc.vector.tensor_scalar_add(pos[:], pos[:], float(TRASH))
        nc.vector.tensor_scalar_min(pos[:], pos[:], float(TMAPN - 1))
        pos_i = gp.tile([128, TC], I32)
        nc.vector.tensor_copy(pos_i[:], pos[:])

        pre = gp.tile([128, 3 * TMAPN // 128], F32)
        nc.vector.memset(pre[:], 0.0)
        for k in range(16):
            nc.sync.dma_start(out=tmds[k].ap().rearrange("(f p) x -> p f x", p=128),
                              in_=pre[:].rearrange("p (f x) -> p f x", x=3))
        for ci in range(TC):
            nc.gpsimd.indirect_dma_start(
                out=tmds[ci % 16][:, :],
                out_offset=bass.IndirectOffsetOnAxis(ap=pos_i[:, ci:ci + 1], axis=0),
                in_=tw[:, 3 * ci:3 * ci + 3], in_offset=None)

        # merge stripes: each slot written by exactly one stripe (ids are +1, zeros elsewhere)
        rg_f = gp.tile([128, CF], F32)   # gather row ids (AG-permuted)
        rt_f = gp.tile([128, CF], F32)   # token row ids (accum order)
        wz = gp.tile([128, CF], F32)
        for k in range(16):
            gpart = gp2.tile([128, CF], F32, tag="gpart", name="gpart")
            nc.sync.dma_start(out=gpart[:],
                              in_=tmds[k].ap()[:C, 0:1].rearrange("(m p) x -> p (m x)", p=128))
            tpart = gp2.tile([128, CF], F32, tag="tpart", name="tpart")
            nc.sync.dma_start(out=tpart[:],
                              in_=tmds[k].ap()[:C, 1:2].rearrange("(m p) x -> p (m x)", p=128))
            wpart = gp2.tile([128, CF], F32, tag="wpart", name="wpart")
            nc.sync.dma_start(out=wpart[:],
                              in_=tmds[k].ap()[:C, 2:3].rearrange("(m p) x -> p (m x)", p=128))
            if k == 0:
                nc.vector.tensor_copy(rg_f[:], gpart[:])
                nc.vector.tensor_copy(rt_f[:], tpart[:])
                nc.vector.tensor_copy(wz[:], wpart[:])
            else:
                nc.vector.tensor_tensor(out=rg_f[:], in0=rg_f[:], in1=gpart[:], op=AOP.add)
                nc.vector.tensor_tensor(out=rt_f[:], in0=rt_f[:], in1=tpart[:], op=AOP.add)
                nc.vector.tensor_tensor(out=wz[:], in0=wz[:], in1=wpart[:], op=AOP.add)
        # ids were scattered as id+1; empty slots (0) -> dummy row T
        for idf in (rg_f, rt_f):
            zeroq = gp2.tile([128, CF], F32, tag="zeroq", name="zeroq")
            nc.vector.tensor_scalar(out=zeroq[:], in0=idf[:], scalar1=0.0, op0=AOP.is_equal, scalar2=0.0, op1=AOP.add)
            nc.vector.tensor_scalar_add(idf[:], idf[:], -1.0)
            nc.vector.tensor_scalar_mul(zeroq[:], zeroq[:], float(T + 1))
            nc.vector.tensor_tensor(out=idf[:], in0=idf[:], in1=zeroq[:], op=AOP.add)
        rg_i = gp.tile([128, CF], I32)
        nc.vector.tensor_copy(rg_i[:], rg_f[:])
        tmap_i = gp.tile([128, CF], I32)
        nc.vector.tensor_copy(tmap_i[:], rt_f[:])

        gath = gp.tile([128, CF, H], F16)
        for m in range(CF):
            nc.gpsimd.indirect_dma_start(
                out=gath[:, m, :], out_offset=None, in_=ag_hs[:, :],
                in_offset=bass.IndirectOffsetOnAxis(ap=rg_i[:, m:m + 1], axis=0))
        hsTc = gp.tile([128, 8, C], F16)
        for m in range(CF):
            for hc in range(8):
                tp = gps.tile([128, 128], F16, space="PSUM", tag="tpg")
                nc.tensor.transpose(out=tp[:], in_=gath[:, m, 128 * hc:128 * (hc + 1)],
                                    identity=c_id16[:])
                nc.scalar.copy(out=hsTc[:, hc, 128 * m:128 * (m + 1)], in_=tp[:])
        gps.release()

        # zero accum rows (all T+128)
        za = gp.tile([128, H], F16)
        nc.vector.memset(za[:], 0.0)
        for i_ in range(T // 128 + 1):
            nc.sync.dma_start(out=accum[128 * i_:128 * (i_ + 1), :], in_=za[:])

        # =========== Phase G: expert FFN over C slots ===========
        fp_ = tc.alloc_tile_pool(name="ffn", bufs=1)
        fp2 = tc.alloc_tile_pool(name="ffn2", bufs=3)
        fps1 = tc.alloc_tile_pool(name="ffn_ps1", bufs=1, space="PSUM")
        fps2 = tc.alloc_tile_pool(name="ffn_ps2", bufs=1, space="PSUM")
        act_blk = fp_.tile([128, IT, TB], F16, tag="act")
        for blk in range(NB):
            cols = slice(TB * blk, TB * (blk + 1))
            for it in range(IT):
                wgS = fp2.tile([128, 8, 128], F16, tag="wgS")
                nc.sync.dma_start(out=wgS[:],
                                  in_=wg.ap()[:, 128 * it:128 * (it + 1)]
                                  .rearrange("(hc p) i -> p hc i", p=128))
                wuS = fp2.tile([128, 8, 128], F16, tag="wuS")
                nc.sync.dma_start(out=wuS[:],
                                  in_=wu.ap()[:, 128 * it:128 * (it + 1)]
                                  .rearrange("(hc p) i -> p hc i", p=128))
                pg = fps1.tile([128, TB], F32, space="PSUM", tag="pg")
                pu = fps1.tile([128, TB], F32, space="PSUM", tag="pu")
                for hc in range(8):
                    nc.tensor.matmul(pg[:], wgS[:, hc, :],
                                     hsTc[:, hc, cols], start=(hc == 0), stop=(hc == 7))
                for hc in range(8):
                    nc.tensor.matmul(pu[:], wuS[:, hc, :],
                                     hsTc[:, hc, cols], start=(hc == 0), stop=(hc == 7))
                sg = fp2.tile([128, TB], F16, tag="sg")
                nc.scalar.activation(sg[:], pg[:], AF.Silu)
                nc.vector.tensor_tensor(out=act_blk[:, it, :], in0=sg[:], in1=pu[:], op=AOP.mult)
            pys = [fps2.tile([128, H], F32, space="PSUM", tag=f"py{ms}", name=f"py{ms}") for ms in range(3)]
            for it in range(IT):
                wdS = fp2.tile([128, H], F16, tag="wdS")
                nc.sync.dma_start(out=wdS[:], in_=wd[128 * it:128 * (it + 1), :])
                for ms in range(3):
                    for half in range(2):
                        nc.tensor.matmul(pys[ms][:, 512 * half:512 * (half + 1)],
                                         act_blk[:, it, 128 * ms:128 * (ms + 1)],
                                         wdS[:, 512 * half:512 * (half + 1)],
                                         start=(it == 0), stop=(it == IT - 1))
            for ms in range(3):
                sidx = 3 * blk + ms
                y16 = fp2.tile([128, H], F16, tag="y16")
                nc.vector.tensor_scalar_mul(y16[:], pys[ms][:], wz[:, sidx:sidx + 1])
                nc.gpsimd.indirect_dma_start(
                    out=accum[:, :],
                    out_offset=bass.IndirectOffsetOnAxis(ap=tmap_i[:, sidx:sidx + 1], axis=0),
                    in_=y16[:], in_offset=None)

        # =========== Phase H: ReduceScatter + final residual ===========
        nc.gpsimd.collective_compute("ReduceScatter", AOP.add, replica_groups=RG,
                                     ins=[accum[0:T, :]], outs=[rs_out[:, :]])
        for qt in range(4):
            rt = fp2.tile([128, H], F16, tag="rt")
            nc.sync.dma_start(out=rt[:], in_=rs_out[128 * qt:128 * (qt + 1), :])
            if _DEBUG:
                moe32 = fp2.tile([128, H], F32, tag="moe32")
                nc.vector.tensor_copy(moe32[:], rt[:])
                nc.sync.dma_start(out=dbg_moe[128 * qt:128 * (qt + 1), :], in_=moe32[:])
            fin = fp2.tile([128, H], F32, tag="fin")
            nc.vector.tensor_tensor(out=fin[:], in0=rt[:], in1=x2[:, qt, :], op=AOP.add)
            nc.sync.dma_start(out=out_dec[128 * qt:128 * (qt + 1), :], in_=fin[:])

        for pool in (fps2, fps1, fp2, fp_, gp2, gp, cp, pp):
            pool.release()

    nc.compile()
    return nc


def _host_inputs(inputs):
    x = np.asarray(inputs["decoder_sequence"], np.float32)        # [B, S, H]
    wq = np.asarray(inputs["wq"], np.float32).reshape(H, NQ * HD) / 8.0
    wk_ = np.asarray(inputs["wk"], np.float32).reshape(H, NKV, HD)
    wk = np.zeros((H, NKV, 2, HD), np.float32)
    wk[:, :, 0] = wk_
    wk[:, :, 1] = wk_
    wk = wk.reshape(H, 2 * NKV * HD)
    wv = np.asarray(inputs["wv"], np.float32).reshape(H, NKV * HD)
    wo = np.asarray(inputs["wo"], np.float32).reshape(NQ * HD, H)
    wr = np.asarray(inputs["w_router"], np.float32)
    wg = np.asarray(inputs["w_gate"], np.float32)
    wu_ = np.asarray(inputs["w_up"], np.float32)
    wd_ = np.asarray(inputs["w_down"], np.float32)

    f16 = lambda a: a.astype(np.float16)

    # constants
    id128 = np.eye(128, dtype=np.float32)
    u128 = np.triu(np.ones((128, 128), np.float16))
    sl32 = np.triu(np.ones((32, 32), np.float32), 1)
    ones_l = np.ones((32, 128), np.float16)
    iota_w = np.zeros((128, 3 * TC), np.float32)
    for ci in range(TC):
        qt_, core_ = ci % 4, ci // 4
        iota_w[:, 3 * ci] = 1024 * qt_ + 128 * core_ + np.arange(128) + 1
        iota_w[:, 3 * ci + 1] = ci * 128 + np.arange(128) + 1
    M = np.zeros((64, 64), np.float32)
    for i_ in range(32):
        M[2 * i_, 32 + i_] = -1.0
        M[2 * i_ + 1, i_] = 1.0
    rot = np.zeros((128, 128), np.float32)
    rot[:64, :64] = M
    rot[64:, 64:] = M
    rotT = rot.T.copy()

    inv = 1.0 / (10000.0 ** (np.arange(0, HD, 2, dtype=np.float64) / HD))  # [32]

    in_maps = []
    for c in range(CORES):
        b, s0 = c // 4, (c % 4) * TPC
        own = x[b, s0:s0 + TPC]                                   # [512, H]
        prev = x[b, s0 - WIN:s0] if s0 >= WIN else np.zeros((WIN, H), np.float32)
        xw = np.concatenate([prev, own], 0)                       # [1024, H]
        posw = np.arange(s0 - WIN, s0 + TPC)
        posc = np.maximum(posw, 0).astype(np.float64)
        emb = posc[None, :] * inv[:, None]                        # [32, 1024]
        cos64 = np.repeat(np.cos(emb), 2, axis=0).astype(np.float32)   # [64, 1024]
        sin64 = np.repeat(np.sin(emb), 2, axis=0).astype(np.float32)
        cosw = np.concatenate([cos64, cos64], 0)                  # [128, 1024]
        sinw = np.concatenate([sin64, sin64], 0)
        m4 = np.full((4, 128, 640), NEG, np.float16)
        ii = np.arange(128)[:, None]
        jj = np.arange(640)[None, :]
        for qt in range(4):
            kg = s0 - WIN + 128 * qt + jj
            valid = (jj > ii) & (jj <= ii + WIN) & (kg >= 0)
            m4[qt][valid.nonzero()] = 0.0
        esel = np.zeros((128, E), np.float32)
        esel[:, c] = 1.0
        in_maps.append({
            "xw16": f16(xw), "xo32": own,
            "wq": f16(wq), "wk": f16(wk), "wv": f16(wv), "wo": f16(wo),
            "wr": wr, "wg": f16(wg[c]), "wu": f16(wu_[c]), "wd": f16(wd_[c]),
            "cosw": cosw, "sinw": sinw, "m4": m4, "rotT": rotT,
            "id16": np.eye(128, dtype=np.float16), "id32": id128,
            "u128": u128, "sl32": sl32, "ones_l": ones_l,
            "iota_w": iota_w, "esel": esel,
        })
    return in_maps


def kernel(**inputs):
    global _nc_cache
    if _nc_cache is None:
        _nc_cache = build_kernel()
    in_maps = _host_inputs(inputs)
    res = run_bass_kernel_spmd(_nc_cache, in_maps, core_ids=list(range(CORES)),
                               trace=bool(int(os.environ.get("BASS_MOE_TRACE", "0"))))
    kernel.last_results = res
    dec = np.concatenate([res.results[c]["out_dec"] for c in range(CORES)], 0)
    rl = np.concatenate([res.results[c]["out_rl"] for c in range(CORES)], 0)
    return dec.reshape(B, S, H).astype(np.float32), rl.astype(np.float32)


# revision 17
# speedup vs baseline: 1.1554x; 1.1554x over previous
"""Mixtral decoder layer (attention + top-2 MoE) on 8 TRN2 NeuronCores.

Sharding: token-parallel attention (512 tokens/core, sliding-window KV
overlap comes in via per-core window inputs), expert-parallel sparse MoE
(1 expert/core, token dispatch via on-device cumsum + indirect DMA
gather/scatter), AllGather of post-attention hidden states + router
logits, ReduceScatter of expert outputs.

Self-contained: hardcodes all shapes; host preprocessing only slices /
casts / builds constant tables.
"""
import os
import numpy as np
import ml_dtypes

import concourse.bass as bass
from concourse import bacc
import concourse.mybir as mybir
import concourse.tile as tile
from concourse.bass_utils import run_bass_kernel_spmd

B, S, H = 2, 2048, 1024
NQ, NKV, HD = 16, 4, 64
I, E, TOP_K = 3584, 8, 2
WIN = 512
EPS = 1e-5
CORES = 8
TPC = 512            # tokens per core
T = B * S            # 4096
TC = T // 128        # 32 token columns for gating math
C = 1152             # expert capacity (9 * 128)
CF = C // 128        # 9 slot tiles
TB = 384             # FFN token-block
NB = C // TB         # 3 blocks
IT = I // 128        # 28 i-tiles
TRASH = C
TMAPN = 1280
NEG = -30000.0

F32, F16, I32 = mybir.dt.float32, mybir.dt.float16, mybir.dt.int32
AOP = mybir.AluOpType
AF = mybir.ActivationFunctionType

_DEBUG = bool(int(os.environ.get("BASS_MOE_DEBUG", "0")))
_nc_cache = None


def build_kernel():
    nc = bacc.Bacc("TRN2", target_bir_lowering=False)
    RG = [list(range(CORES))]

    # ---------------- parameters ----------------
    xw16 = nc.declare_dram_parameter("xw16", [1024, H], F16, isOutput=False)
    xo32 = nc.declare_dram_parameter("xo32", [TPC, H], F32, isOutput=False)
    wq = nc.declare_dram_parameter("wq", [H, NQ * HD], F16, isOutput=False)
    wk = nc.declare_dram_parameter("wk", [H, 2 * NKV * HD], F16, isOutput=False)
    wv = nc.declare_dram_parameter("wv", [H, NKV * HD], F16, isOutput=False)
    wo = nc.declare_dram_parameter("wo", [NQ * HD, H], F16, isOutput=False)
    wr = nc.declare_dram_parameter("wr", [H, E], F32, isOutput=False)
    wg = nc.declare_dram_parameter("wg", [H, I], F16, isOutput=False)
    wu = nc.declare_dram_parameter("wu", [H, I], F16, isOutput=False)
    wd = nc.declare_dram_parameter("wd", [I, H], F16, isOutput=False)
    cosw = nc.declare_dram_parameter("cosw", [128, 1024], F32, isOutput=False)
    sinw = nc.declare_dram_parameter("sinw", [128, 1024], F32, isOutput=False)
    m4 = nc.declare_dram_parameter("m4", [4, 128, 640], F16, isOutput=False)
    rotT = nc.declare_dram_parameter("rotT", [128, 128], F32, isOutput=False)
    id16 = nc.declare_dram_parameter("id16", [128, 128], F16, isOutput=False)
    id32 = nc.declare_dram_parameter("id32", [128, 128], F32, isOutput=False)
    u128 = nc.declare_dram_parameter("u128", [128, 128], F16, isOutput=False)
    sl32 = nc.declare_dram_parameter("sl32", [32, 32], F32, isOutput=False)
    ones_l = nc.declare_dram_parameter("ones_l", [32, 128], F16, isOutput=False)
    iota_w = nc.declare_dram_parameter("iota_w", [128, 2 * TC], F32, isOutput=False)
    esel = nc.declare_dram_parameter("esel", [128, E], F32, isOutput=False)

    out_dec = nc.declare_dram_parameter("out_dec", [TPC, H], F32, isOutput=True)
    out_rl = nc.declare_dram_parameter("out_rl", [TPC, E], F32, isOutput=True)
    if _DEBUG:
        dbg_x2 = nc.declare_dram_parameter("dbg_x2", [TPC, H], F32, isOutput=True)
        dbg_moe = nc.declare_dram_parameter("dbg_moe", [TPC, H], F32, isOutput=True)

    # ---------------- internal DRAM ----------------
    ag_in = nc.dram_tensor("ag_in", [TPC, H], F16)
    ag_hs = nc.dram_tensor("ag_hs", [T + 128, H], F16, addr_space="Shared")
    lg_in = nc.dram_tensor("lg_in", [TPC, E], F32)
    ag_lg = nc.dram_tensor("ag_lg", [T, E], F32, addr_space="Shared")
    tmds = [nc.dram_tensor(f"tmap_dram{k}", [TMAPN, 2], F32) for k in range(8)]
    accum = nc.dram_tensor("accum", [T + 128, H], F16)
    rs_out = nc.dram_tensor("rs_out", [TPC, H], F16)

    with tile.TileContext(nc) as tc:
        # persistent pool (consts + tensors alive across phases)
        pp = tc.alloc_tile_pool(name="persist", bufs=1)
        c_id16 = pp.tile([128, 128], F16)
        c_id32 = pp.tile([128, 128], F32)
        c_u128 = pp.tile([128, 128], F16)
        c_sl32 = pp.tile([32, 32], F32)
        c_onesl = pp.tile([32, 128], F16)
        c_rotT = pp.tile([128, 128], F32)
        c_esel = pp.tile([128, E], F32)
        tw = pp.tile([128, 2 * TC], F32)
        nc.sync.dma_start(out=c_id16[:], in_=id16[:, :])
        nc.sync.dma_start(out=c_id32[:], in_=id32[:, :])
        nc.sync.dma_start(out=c_u128[:], in_=u128[:, :])
        nc.sync.dma_start(out=c_sl32[:], in_=sl32[:, :])
        nc.sync.dma_start(out=c_onesl[:], in_=ones_l[:, :])
        nc.sync.dma_start(out=c_rotT[:], in_=rotT[:, :])
        nc.sync.dma_start(out=c_esel[:], in_=esel[:, :])
        nc.sync.dma_start(out=tw[:], in_=iota_w[:, :])
        x2 = pp.tile([128, 4, H], F32)          # post-attention + residual (own tokens)
        logit_sb = pp.tile([128, 4, E], F32)    # own-slice router logits

        # =========== Phase A/B/C: attention ===========
        ap = tc.alloc_tile_pool(name="attn", bufs=1)
        ap1 = tc.alloc_tile_pool(name="attn1", bufs=1)
        ap2 = tc.alloc_tile_pool(name="attn2", bufs=2)
        aps = tc.alloc_tile_pool(name="attn_ps", bufs=1, space="PSUM")
        aps2 = tc.alloc_tile_pool(name="attn_ps2", bufs=2, space="PSUM")

        c_cos = ap.tile([128, 1024], F32)
        c_sin = ap.tile([128, 1024], F32)
        c_m4 = ap.tile([128, 4, 640], F16)
        nc.sync.dma_start(out=c_cos[:], in_=cosw[:, :])
        nc.sync.dma_start(out=c_sin[:], in_=sinw[:, :])
        nc.sync.dma_start(out=c_m4[:], in_=m4.ap().rearrange("q p k -> p q k"))
        wq_sb = ap.tile([128, 8, NQ * HD], F16)
        wk_sb = ap.tile([128, 8, 2 * NKV * HD], F16)
        wv_sb = ap.tile([128, 8, NKV * HD], F16)
        wo_sb = ap.tile([128, 8, H], F16)
        nc.sync.dma_start(out=wq_sb[:], in_=wq.ap().rearrange("(hc p) n -> p hc n", p=128))
        nc.sync.dma_start(out=wk_sb[:], in_=wk.ap().rearrange("(hc p) n -> p hc n", p=128))
        nc.sync.dma_start(out=wv_sb[:], in_=wv.ap().rearrange("(hc p) n -> p hc n", p=128))
        nc.sync.dma_start(out=wo_sb[:], in_=wo.ap().rearrange("(nc p) h -> p nc h", p=128))

        # A: rmsnorm(x_win) -> hs (token-major), then transpose -> hsT
        hsT = ap.tile([128, 8, 1024], F16)      # [h%128, h//128, window tok]
        for tt in range(8):
            xt = ap2.tile([128, H], F16, tag="xw")
            nc.sync.dma_start(out=xt[:], in_=xw16[128 * tt:128 * (tt + 1), :])
            sq = ap1.tile([128, H], F32, tag="sq")
            nc.vector.tensor_tensor(out=sq[:], in0=xt[:], in1=xt[:], op=AOP.mult)
            ssum = ap2.tile([128, 1], F32, tag="ssum")
            nc.vector.tensor_reduce(out=ssum[:], in_=sq[:], axis=mybir.AxisListType.X, op=AOP.add)
            nc.vector.tensor_scalar(out=ssum[:], in0=ssum[:], scalar1=1.0 / H, op0=AOP.mult,
                                    scalar2=EPS, op1=AOP.add)
            nc.vector.reciprocal(out=ssum[:], in_=ssum[:])
            rs_ = ap2.tile([128, 1], F32, tag="rs")
            nc.scalar.activation(rs_[:], ssum[:], AF.Sqrt)
            hst = ap2.tile([128, H], F16, tag="hst")
            nc.vector.tensor_scalar_mul(hst[:], xt[:], rs_[:, :1])
            for hc in range(8):
                tp = aps2.tile([128, 128], F16, space="PSUM", tag="tp")
                nc.tensor.transpose(out=tp[:], in_=hst[:, 128 * hc:128 * (hc + 1)], identity=c_id16[:])
                nc.scalar.copy(out=hsT[:, hc, 128 * tt:128 * (tt + 1)], in_=tp[:])

        # B: QKV projections + rope
        qroT = ap.tile([128, 8, 512], F16)      # [(2 heads)*64, m, q tok]
        kroT = ap.tile([128, 4, 1024], F16)
        v16 = ap.tile([128, 8, NKV * HD], F16)  # token-major V
        for m in range(8):
            pq = aps.tile([128, 512], F32, space="PSUM", tag="med")
            for hc in range(8):
                nc.tensor.matmul(pq[:], wq_sb[:, hc, 128 * m:128 * (m + 1)],
                                 hsT[:, hc, 512:1024], start=(hc == 0), stop=(hc == 7))
            qsb = ap2.tile([128, 512], F32, tag="qsb")
            nc.vector.tensor_copy(qsb[:], pq[:])
            prot = aps.tile([128, 512], F32, space="PSUM", tag="med")
            nc.tensor.matmul(prot[:], c_rotT[:], qsb[:], start=True, stop=True)
            t1 = ap2.tile([128, 512], F32, tag="t1")
            nc.vector.tensor_tensor(out=t1[:], in0=qsb[:], in1=c_cos[:, 512:1024], op=AOP.mult)
            t2 = ap2.tile([128, 512], F32, tag="t2")
            nc.vector.tensor_copy(t2[:], prot[:])
            nc.vector.tensor_tensor(out=t2[:], in0=t2[:], in1=c_sin[:, 512:1024], op=AOP.mult)
            nc.vector.tensor_tensor(out=qroT[:, m, :], in0=t1[:], in1=t2[:], op=AOP.add)
        for m in range(4):
            pk = aps.tile([128, 1024], F32, space="PSUM", tag="big")
            for half in range(2):
                for hc in range(8):
                    nc.tensor.matmul(pk[:, 512 * half:512 * (half + 1)],
                                     wk_sb[:, hc, 128 * m:128 * (m + 1)],
                                     hsT[:, hc, 512 * half:512 * (half + 1)],
                                     start=(hc == 0), stop=(hc == 7))
            ksb = ap1.tile([128, 1024], F32, tag="ksb")
            nc.vector.tensor_copy(ksb[:], pk[:])
            prot = aps.tile([128, 1024], F32, space="PSUM", tag="big")
            nc.tensor.matmul(prot[:, 0:512], c_rotT[:], ksb[:, 0:512], start=True, stop=True)
            nc.tensor.matmul(prot[:, 512:1024], c_rotT[:], ksb[:, 512:1024], start=True, stop=True)
            t1 = ap1.tile([128, 1024], F32, tag="t1k")
            nc.vector.tensor_tensor(out=t1[:], in0=ksb[:], in1=c_cos[:], op=AOP.mult)
            t2 = ap1.tile([128, 1024], F32, tag="t2k")
            nc.vector.tensor_copy(t2[:], prot[:])
            nc.vector.tensor_tensor(out=t2[:], in0=t2[:], in1=c_sin[:], op=AOP.mult)
            nc.vector.tensor_tensor(out=kroT[:, m, :], in0=t1[:], in1=t2[:], op=AOP.add)
        for tt in range(8):
            pv = aps.tile([128, NKV * HD], F32, space="PSUM", tag="med")
            for hc in range(8):
                nc.tensor.matmul(pv[:], hsT[:, hc, 128 * tt:128 * (tt + 1)],
                                 wv_sb[:, hc, :], start=(hc == 0), stop=(hc == 7))
            nc.vector.tensor_copy(v16[:, tt, :], pv[:])

        # C: per q-tile attention
        wr_sb = ap.tile([128, 8, E], F32)
        nc.sync.dma_start(out=wr_sb[:], in_=wr.ap().rearrange("(hc p) e -> p hc e", p=128))
        for qt in range(4):
            acat = ap.tile([128, 8, 128], F16, tag="acat")
            for kh in range(NKV):
                for ql in range(4):
                    qh = 4 * kh + ql
                    ps = aps.tile([128, 640], F32, space="PSUM", tag="score")
                    par = 64 * (qh % 2)
                    lq = qroT[par:par + 64, qh // 2, 128 * qt:128 * (qt + 1)]
                    rk = kroT[par:par + 64, kh, :]
                    nc.tensor.matmul(ps[:, 0:512], lq, rk[:, 128 * qt:128 * qt + 512],
                                     start=True, stop=True)
                    nc.tensor.matmul(ps[:, 512:640], lq, rk[:, 128 * qt + 512:128 * qt + 640],
                                     start=True, stop=True)
                    nc.vector.tensor_tensor(out=ps[:], in0=ps[:], in1=c_m4[:, qt, :], op=AOP.add)
                    mx = ap2.tile([128, 1], F32, tag="mx")
                    nc.vector.tensor_reduce(out=mx[:], in_=ps[:], axis=mybir.AxisListType.X, op=AOP.max)
                    nc.vector.tensor_scalar_mul(mx[:], mx[:], -1.0)
                    pr = ap2.tile([128, 640], F16, tag="pr")
                    nc.scalar.activation(pr[:], ps[:], AF.Exp, bias=mx[:, :1])
                    se = ap2.tile([128, 1], F32, tag="se")
                    nc.vector.tensor_reduce(out=se[:], in_=pr[:], axis=mybir.AxisListType.X, op=AOP.add)
                    nc.vector.reciprocal(out=se[:], in_=se[:])
                    nc.vector.tensor_scalar_mul(pr[:], pr[:], se[:, :1])
                    prT = ap2.tile([128, 5, 128], F16, tag="prT")
                    for kc in range(5):
                        tp = aps2.tile([128, 128], F16, space="PSUM", tag="tp")
                        nc.tensor.transpose(out=tp[:], in_=pr[:, 128 * kc:128 * (kc + 1)],
                                            identity=c_id16[:])
                        nc.scalar.copy(out=prT[:, kc, :], in_=tp[:])
                    pa = aps.tile([64, 128], F32, space="PSUM", tag="med")
                    for kc in range(5):
                        nc.tensor.matmul(pa[:], v16[:, qt + kc, 64 * kh:64 * kh + 64],
                                         prT[:, kc, :], start=(kc == 0), stop=(kc == 4))
                    patmp = ap2.tile([64, 128], F16, tag="patmp")
                    nc.scalar.copy(out=patmp[:], in_=pa[:])
                    nc.sync.dma_start(out=acat[64 * (qh % 2):64 * (qh % 2) + 64, qh // 2, :],
                                      in_=patmp[:])
            po = aps.tile([128, 1024], F32, space="PSUM", tag="big")
            for half in range(2):
                for b in range(8):
                    nc.tensor.matmul(po[:, 512 * half:512 * (half + 1)], acat[:, b, :],
                                     wo_sb[:, b, 512 * half:512 * (half + 1)],
                                     start=(b == 0), stop=(b == 7))
            xo_t = ap1.tile([128, H], F32, tag="xo")
            nc.sync.dma_start(out=xo_t[:], in_=xo32[128 * qt:128 * (qt + 1), :])
            nc.vector.tensor_tensor(out=x2[:, qt, :], in0=po[:], in1=xo_t[:], op=AOP.add)

            # norm2 + hs2 (fp16, to AG) + hs2T (f32, router)
            sq = ap1.tile([128, H], F32, tag="sq2")
            nc.vector.tensor_tensor(out=sq[:], in0=x2[:, qt, :], in1=x2[:, qt, :], op=AOP.mult)
            ssum = ap2.tile([128, 1], F32, tag="ssum2")
            nc.vector.tensor_reduce(out=ssum[:], in_=sq[:], axis=mybir.AxisListType.X, op=AOP.add)
            nc.vector.tensor_scalar(out=ssum[:], in0=ssum[:], scalar1=1.0 / H, op0=AOP.mult,
                                    scalar2=EPS, op1=AOP.add)
            nc.vector.reciprocal(out=ssum[:], in_=ssum[:])
            rs_ = ap2.tile([128, 1], F32, tag="rs2")
            nc.scalar.activation(rs_[:], ssum[:], AF.Sqrt)
            hs2_32 = ap1.tile([128, H], F32, tag="hs2_32")
            nc.vector.tensor_scalar_mul(hs2_32[:], x2[:, qt, :], rs_[:, :1])
            hs2_16 = ap2.tile([128, H], F16, tag="hs2_16")
            nc.vector.tensor_copy(hs2_16[:], hs2_32[:])
            nc.sync.dma_start(out=ag_in[128 * qt:128 * (qt + 1), :], in_=hs2_16[:])
            # router: logits = hs2 @ wr  (f32; lhsT = hs2T chunks)
            plg = aps.tile([128, E], F32, space="PSUM", tag="med")
            for hc in range(8):
                tp32 = aps2.tile([128, 128], F32, space="PSUM", tag="tp")
                nc.tensor.transpose(out=tp32[:], in_=hs2_32[:, 128 * hc:128 * (hc + 1)],
                                    identity=c_id32[:])
                h2T = ap2.tile([128, 128], F32, tag="h2T")
                nc.vector.tensor_copy(h2T[:], tp32[:])
                nc.tensor.matmul(plg[:], h2T[:], wr_sb[:, hc, :], start=(hc == 0), stop=(hc == 7))
            nc.vector.tensor_copy(logit_sb[:, qt, :], plg[:])
            nc.sync.dma_start(out=lg_in[128 * qt:128 * (qt + 1), :], in_=logit_sb[:, qt, :])
            nc.sync.dma_start(out=out_rl[128 * qt:128 * (qt + 1), :], in_=logit_sb[:, qt, :])
            if _DEBUG:
                nc.sync.dma_start(out=dbg_x2[128 * qt:128 * (qt + 1), :], in_=x2[:, qt, :])

        for pool in (aps2, aps, ap2, ap1, ap):
            pool.release()

        # =========== AG collectives ===========
        cp = tc.alloc_tile_pool(name="coll", bufs=1)
        zt = cp.tile([128, H], F16)
        nc.vector.memset(zt[:], 0.0)
        nc.sync.dma_start(out=ag_hs[T:T + 128, :], in_=zt[:])
        nc.gpsimd.collective_compute("AllGather", AOP.bypass, replica_groups=RG,
                                     ins=[ag_in[:, :]], outs=[ag_hs[0:T, :]])
        nc.gpsimd.collective_compute("AllGather", AOP.bypass, replica_groups=RG,
                                     ins=[lg_in[:, :]], outs=[ag_lg[:, :]])

        # =========== Phase E: gating over all T tokens ===========
        gp = tc.alloc_tile_pool(name="gate", bufs=1)
        gp2 = tc.alloc_tile_pool(name="gate2", bufs=3)
        gps = tc.alloc_tile_pool(name="gate_ps", bufs=2, space="PSUM")
        mask32 = gp.tile([128, TC], F32)
        lt_all = gp.tile([128, TC, E], F32)
        nc.sync.dma_start(out=lt_all[:], in_=ag_lg.ap().rearrange("(c p) e -> p c e", p=128))
        m1 = gp.tile([128, TC], F32)
        nc.vector.tensor_reduce(out=m1[:], in_=lt_all[:], axis=mybir.AxisListType.X, op=AOP.max)
        m1b = m1[:].to_broadcast([128, TC, E])
        iseq = gp.tile([128, TC, E], F32)
        nc.vector.tensor_tensor(out=iseq[:], in0=lt_all[:], in1=m1b, op=AOP.is_equal)
        lt2 = gp.tile([128, TC, E], F32)
        nc.vector.scalar_tensor_tensor(out=lt2[:], in0=iseq[:], scalar=-1e9, in1=lt_all[:],
                                       op0=AOP.mult, op1=AOP.add)
        m2 = gp.tile([128, TC], F32)
        nc.vector.tensor_reduce(out=m2[:], in_=lt2[:], axis=mybir.AxisListType.X, op=AOP.max)
        eselb = c_esel[:].rearrange("p e -> p () e").to_broadcast([128, TC, E])
        lesel = gp.tile([128, TC, E], F32)
        nc.vector.tensor_tensor(out=lesel[:], in0=lt_all[:], in1=eselb, op=AOP.mult)
        le = gp.tile([128, TC], F32)
        nc.vector.tensor_reduce(out=le[:], in_=lesel[:], axis=mybir.AxisListType.X, op=AOP.add)
        nc.vector.tensor_tensor(out=mask32[:], in0=le[:], in1=m2[:], op=AOP.is_ge)
        d21 = gp.tile([128, TC], F32)
        nc.vector.tensor_tensor(out=d21[:], in0=m2[:], in1=m1[:], op=AOP.subtract)
        e2 = gp.tile([128, TC], F32)
        nc.scalar.activation(e2[:], d21[:], AF.Exp)
        nc.vector.tensor_scalar_add(e2[:], e2[:], 1.0)
        nc.vector.reciprocal(out=e2[:], in_=e2[:])
        dle = gp.tile([128, TC], F32)
        nc.vector.tensor_tensor(out=dle[:], in0=le[:], in1=m1[:], op=AOP.subtract)
        ew = gp.tile([128, TC], F32)
        nc.scalar.activation(ew[:], dle[:], AF.Exp)
        nc.vector.tensor_tensor(out=ew[:], in0=ew[:], in1=e2[:], op=AOP.mult)
        nc.vector.tensor_tensor(out=ew[:], in0=ew[:], in1=mask32[:], op=AOP.mult)
        nc.vector.tensor_copy(tw[:, 1:2 * TC:2], ew[:])

        # =========== Phase F: dispatch ===========
        mask16 = gp.tile([128, TC], F16)
        nc.vector.tensor_copy(mask16[:], mask32[:])
        colcum = gps.tile([128, TC], F32, space="PSUM", tag="colcum")
        nc.tensor.matmul(colcum[:], c_u128[:], mask16[:], start=True, stop=True)
        ones128 = gp.tile([128, 1], F16)
        nc.vector.memset(ones128[:], 1.0)
        counts_p = gps.tile([32, 1], F32, space="PSUM", tag="counts")
        nc.tensor.matmul(counts_p[:], mask16[:], ones128[:], start=True, stop=True)
        counts = gp.tile([32, 1], F32)
        nc.vector.tensor_copy(counts[:], counts_p[:])
        slc = gp.tile([32, 32], F16)
        nc.vector.tensor_scalar_mul(slc[:], c_sl32[:], counts[:, :1])
        carry = gps.tile([128, TC], F32, space="PSUM", tag="carry")
        nc.tensor.matmul(carry[:], c_onesl[:], slc[:, :TC], start=True, stop=True)
        pos = gp.tile([128, TC], F32)
        nc.vector.tensor_copy(pos[:], colcum[:])
        nc.vector.tensor_tensor(out=pos[:], in0=pos[:], in1=carry[:], op=AOP.add)
        nc.vector.tensor_tensor(out=pos[:], in0=pos[:], in1=mask32[:], op=AOP.subtract)
        nc.vector.tensor_scalar_add(pos[:], pos[:], -float(TRASH))
        nc.vector.tensor_tensor(out=pos[:], in0=pos[:], in1=mask32[:], op=AOP.mult)
        nc.vector.tensor_scalar_add(pos[:], pos[:], float(TRASH))
        nc.vector.tensor_scalar_min(pos[:], pos[:], float(TMAPN - 1))
        pos_i = gp.tile([128, TC], I32)
        nc.vector.tensor_copy(pos_i[:], pos[:])

        pre = gp.tile([128, 2 * TMAPN // 128], F32)
        nc.vector.memset(pre[:], 0.0)
        for k in range(8):
            nc.sync.dma_start(out=tmds[k].ap().rearrange("(f p) x -> p f x", p=128),
                              in_=pre[:].rearrange("p (f x) -> p f x", x=2))
        for ci in range(TC):
            nc.gpsimd.indirect_dma_start(
                out=tmds[ci % 8][:, :],
                out_offset=bass.IndirectOffsetOnAxis(ap=pos_i[:, ci:ci + 1], axis=0),
                in_=tw[:, 2 * ci:2 * ci + 2], in_offset=None)

        # merge stripes: each slot written by exactly one stripe (ids are +1, zeros elsewhere)
        tmap_f = gp.tile([128, CF], F32)
        wz = gp.tile([128, CF], F32)
        for k in range(8):
            tpart = gp2.tile([128, CF], F32, tag="tpart", name="tpart")
            nc.sync.dma_start(out=tpart[:],
                              in_=tmds[k].ap()[:C, 0:1].rearrange("(m p) x -> p (m x)", p=128))
            wpart = gp2.tile([128, CF], F32, tag="wpart", name="wpart")
            nc.sync.dma_start(out=wpart[:],
                              in_=tmds[k].ap()[:C, 1:2].rearrange("(m p) x -> p (m x)", p=128))
            if k == 0:
                nc.vector.tensor_copy(tmap_f[:], tpart[:])
                nc.vector.tensor_copy(wz[:], wpart[:])
            else:
                nc.vector.tensor_tensor(out=tmap_f[:], in0=tmap_f[:], in1=tpart[:], op=AOP.add)
                nc.vector.tensor_tensor(out=wz[:], in0=wz[:], in1=wpart[:], op=AOP.add)
        # ids were scattered as id+1; empty slots (0) -> dummy token T
        zeroq = gp.tile([128, CF], F32)
        nc.vector.tensor_scalar(out=zeroq[:], in0=tmap_f[:], scalar1=0.0, op0=AOP.is_equal, scalar2=0.0, op1=AOP.add)
        nc.vector.tensor_scalar_add(tmap_f[:], tmap_f[:], -1.0)
        nc.vector.tensor_scalar_mul(zeroq[:], zeroq[:], float(T + 1))
        nc.vector.tensor_tensor(out=tmap_f[:], in0=tmap_f[:], in1=zeroq[:], op=AOP.add)
        tmap_i = gp.tile([128, CF], I32)
        nc.vector.tensor_copy(tmap_i[:], tmap_f[:])

        gath = gp.tile([128, CF, H], F16)
        for m in range(CF):
            nc.gpsimd.indirect_dma_start(
                out=gath[:, m, :], out_offset=None, in_=ag_hs[:, :],
                in_offset=bass.IndirectOffsetOnAxis(ap=tmap_i[:, m:m + 1], axis=0))
        hsTc = gp.tile([128, 8, C], F16)
        for m in range(CF):
            for hc in range(8):
                tp = gps.tile([128, 128], F16, space="PSUM", tag="tpg")
                nc.tensor.transpose(out=tp[:], in_=gath[:, m, 128 * hc:128 * (hc + 1)],
                                    identity=c_id16[:])
                nc.scalar.copy(out=hsTc[:, hc, 128 * m:128 * (m + 1)], in_=tp[:])
        gps.release()

        # zero accum rows (all T+128)
        za = gp.tile([128, H], F16)
        nc.vector.memset(za[:], 0.0)
        for i_ in range(T // 128 + 1):
            nc.sync.dma_start(out=accum[128 * i_:128 * (i_ + 1), :], in_=za[:])

        # =========== Phase G: expert FFN over C slots ===========
        fp_ = tc.alloc_tile_pool(name="ffn", bufs=1)
        fp2 = tc.alloc_tile_pool(name="ffn2", bufs=3)
        fps1 = tc.alloc_tile_pool(name="ffn_ps1", bufs=1, space="PSUM")
        fps2 = tc.alloc_tile_pool(name="ffn_ps2", bufs=1, space="PSUM")
        act_blk = fp_.tile([128, IT, TB], F16, tag="act")
        for blk in range(NB):
            cols = slice(TB * blk, TB * (blk + 1))
            for it in range(IT):
                wgS = fp2.tile([128, 8, 128], F16, tag="wgS")
                nc.sync.dma_start(out=wgS[:],
                                  in_=wg.ap()[:, 128 * it:128 * (it + 1)]
                                  .rearrange("(hc p) i -> p hc i", p=128))
                wuS = fp2.tile([128, 8, 128], F16, tag="wuS")
                nc.sync.dma_start(out=wuS[:],
                                  in_=wu.ap()[:, 128 * it:128 * (it + 1)]
                                  .rearrange("(hc p) i -> p hc i", p=128))
                pg = fps1.tile([128, TB], F32, space="PSUM", tag="pg")
                pu = fps1.tile([128, TB], F32, space="PSUM", tag="pu")
                for hc in range(8):
                    nc.tensor.matmul(pg[:], wgS[:, hc, :],
                                     hsTc[:, hc, cols], start=(hc == 0), stop=(hc == 7))
                for hc in range(8):
                    nc.tensor.matmul(pu[:], wuS[:, hc, :],
                                     hsTc[:, hc, cols], start=(hc == 0), stop=(hc == 7))
                sg = fp2.tile([128, TB], F16, tag="sg")
                nc.scalar.activation(sg[:], pg[:], AF.Silu)
                nc.vector.tensor_tensor(out=act_blk[:, it, :], in0=sg[:], in1=pu[:], op=AOP.mult)
            pys = [fps2.tile([128, H], F32, space="PSUM", tag=f"py{ms}", name=f"py{ms}") for ms in range(3)]
            for it in range(IT):
                wdS = fp2.tile([128, H], F16, tag="wdS")
                nc.sync.dma_start(out=wdS[:], in_=wd[128 * it:128 * (it + 1), :])
                for ms in range(3):
                    for half in range(2):
                        nc.tensor.matmul(pys[ms][:, 512 * half:512 * (half + 1)],
                                         act_blk[:, it, 128 * ms:128 * (ms + 1)],
                                         wdS[:, 512 * half:512 * (half + 1)],
                                         start=(it == 0), stop=(it == IT - 1))
            for ms in range(3):
                sidx = 3 * blk + ms
                y16 = fp2.tile([128, H], F16, tag="y16")
                nc.vector.tensor_scalar_mul(y16[:], pys[ms][:], wz[:, sidx:sidx + 1])
                nc.gpsimd.indirect_dma_start(
                    out=accum[:, :],
                    out_offset=bass.IndirectOffsetOnAxis(ap=tmap_i[:, sidx:sidx + 1], axis=0),
                    in_=y16[:], in_offset=None)

        # =========== Phase H: ReduceScatter + final residual ===========
        nc.gpsimd.collective_compute("ReduceScatter", AOP.add, replica_groups=RG,
                                     ins=[accum[0:T, :]], outs=[rs_out[:, :]])
        for qt in range(4):
            rt = fp2.tile([128, H], F16, tag="rt")
            nc.sync.dma_start(out=rt[:], in_=rs_out[128 * qt:128 * (qt + 1), :])
            if _DEBUG:
                moe32 = fp2.tile([128, H], F32, tag="moe32")
                nc.vector.tensor_copy(moe32[:], rt[:])
                nc.sync.dma_start(out=dbg_moe[128 * qt:128 * (qt + 1), :], in_=moe32[:])
            fin = fp2.tile([128, H], F32, tag="fin")
            nc.vector.tensor_tensor(out=fin[:], in0=rt[:], in1=x2[:, qt, :], op=AOP.add)
            nc.sync.dma_start(out=out_dec[128 * qt:128 * (qt + 1), :], in_=fin[:])

        for pool in (fps2, fps1, fp2, fp_, gp2, gp, cp, pp):
            pool.release()

    nc.compile()
    return nc


def _host_inputs(inputs):
    x = np.asarray(inputs["decoder_sequence"], np.float32)        # [B, S, H]
    wq = np.asarray(inputs["wq"], np.float32).reshape(H, NQ * HD) / 8.0
    wk_ = np.asarray(inputs["wk"], np.float32).reshape(H, NKV, HD)
    wk = np.zeros((H, NKV, 2, HD), np.float32)
    wk[:, :, 0] = wk_
    wk[:, :, 1] = wk_
    wk = wk.reshape(H, 2 * NKV * HD)
    wv = np.asarray(inputs["wv"], np.float32).reshape(H, NKV * HD)
    wo = np.asarray(inputs["wo"], np.float32).reshape(NQ * HD, H)
    wr = np.asarray(inputs["w_router"], np.float32)
    wg = np.asarray(inputs["w_gate"], np.float32)
    wu_ = np.asarray(inputs["w_up"], np.float32)
    wd_ = np.asarray(inputs["w_down"], np.float32)

    f16 = lambda a: a.astype(np.float16)

    # constants
    id128 = np.eye(128, dtype=np.float32)
    u128 = np.triu(np.ones((128, 128), np.float16))
    sl32 = np.triu(np.ones((32, 32), np.float32), 1)
    ones_l = np.ones((32, 128), np.float16)
    iota_w = np.zeros((128, 2 * TC), np.float32)
    for ci in range(TC):
        iota_w[:, 2 * ci] = ci * 128 + np.arange(128) + 1
    M = np.zeros((64, 64), np.float32)
    for i_ in range(32):
        M[2 * i_, 32 + i_] = -1.0
        M[2 * i_ + 1, i_] = 1.0
    rot = np.zeros((128, 128), np.float32)
    rot[:64, :64] = M
    rot[64:, 64:] = M
    rotT = rot.T.copy()

    inv = 1.0 / (10000.0 ** (np.arange(0, HD, 2, dtype=np.float64) / HD))  # [32]

    in_maps = []
    for c in range(CORES):
        b, s0 = c // 4, (c % 4) * TPC
        own = x[b, s0:s0 + TPC]                                   # [512, H]
        prev = x[b, s0 - WIN:s0] if s0 >= WIN else np.zeros((WIN, H), np.float32)
        xw = np.concatenate([prev, own], 0)                       # [1024, H]
        posw = np.arange(s0 - WIN, s0 + TPC)
        posc = np.maximum(posw, 0).astype(np.float64)
        emb = posc[None, :] * inv[:, None]                        # [32, 1024]
        cos64 = np.repeat(np.cos(emb), 2, axis=0).astype(np.float32)   # [64, 1024]
        sin64 = np.repeat(np.sin(emb), 2, axis=0).astype(np.float32)
        cosw = np.concatenate([cos64, cos64], 0)                  # [128, 1024]
        sinw = np.concatenate([sin64, sin64], 0)
        m4 = np.full((4, 128, 640), NEG, np.float16)
        ii = np.arange(128)[:, None]
        jj = np.arange(640)[None, :]
        for qt in range(4):
            kg = s0 - WIN + 128 * qt + jj
            valid = (jj > ii) & (jj <= ii + WIN) & (kg >= 0)
            m4[qt][valid.nonzero()] = 0.0
        esel = np.zeros((128, E), np.float32)
        esel[:, c] = 1.0
        in_maps.append({
            "xw16": f16(xw), "xo32": own,
            "wq": f16(wq), "wk": f16(wk), "wv": f16(wv), "wo": f16(wo),
            "wr": wr, "wg": f16(wg[c]), "wu": f16(wu_[c]), "wd": f16(wd_[c]),
            "cosw": cosw, "sinw": sinw, "m4": m4, "rotT": rotT,
            "id16": np.eye(128, dtype=np.float16), "id32": id128,
            "u128": u128, "sl32": sl32, "ones_l": ones_l,
            "iota_w": iota_w, "esel": esel,
        })
    return in_maps


def kernel(**inputs):
    global _nc_cache
    if _nc_cache is None:
        _nc_cache = build_kernel()
    in_maps = _host_inputs(inputs)
    res = run_bass_kernel_spmd(_nc_cache, in_maps, core_ids=list(range(CORES)),
                               trace=bool(int(os.environ.get("BASS_MOE_TRACE", "0"))))
    kernel.last_results = res
    dec = np.concatenate([res.results[c]["out_dec"] for c in range(CORES)], 0)
    rl = np.concatenate([res.results[c]["out_rl"] for c in range(CORES)], 0)
    return dec.reshape(B, S, H).astype(np.float32), rl.astype(np.float32)


# revision 19
# speedup vs baseline: 1.1810x; 1.0222x over previous
"""Mixtral decoder layer (attention + top-2 MoE) on 8 TRN2 NeuronCores.

Sharding: token-parallel attention (512 tokens/core, sliding-window KV
overlap comes in via per-core window inputs), expert-parallel sparse MoE
(1 expert/core, token dispatch via on-device cumsum + indirect DMA
gather/scatter), AllGather of post-attention hidden states + router
logits, ReduceScatter of expert outputs.

Self-contained: hardcodes all shapes; host preprocessing only slices /
casts / builds constant tables.
"""
import os
import numpy as np
import ml_dtypes

import concourse.bass as bass
from concourse import bacc
import concourse.mybir as mybir
import concourse.tile as tile
from concourse.bass_utils import run_bass_kernel_spmd

B, S, H = 2, 2048, 1024
NQ, NKV, HD = 16, 4, 64
I, E, TOP_K = 3584, 8, 2
WIN = 512
EPS = 1e-5
CORES = 8
TPC = 512            # tokens per core
T = B * S            # 4096
TC = T // 128        # 32 token columns for gating math
C = 1152             # expert capacity (9 * 128)
CF = C // 128        # 9 slot tiles
TB = 384             # FFN token-block
NB = C // TB         # 3 blocks
IT = I // 128        # 28 i-tiles
TRASH = C
TMAPN = 1280
NEG = -30000.0

F32, F16, I32 = mybir.dt.float32, mybir.dt.float16, mybir.dt.int32
AOP = mybir.AluOpType
AF = mybir.ActivationFunctionType

_DEBUG = bool(int(os.environ.get("BASS_MOE_DEBUG", "0")))
_nc_cache = None


def build_kernel():
    nc = bacc.Bacc("TRN2", target_bir_lowering=False)
    RG = [list(range(CORES))]

    # ---------------- parameters ----------------
    xw16 = nc.declare_dram_parameter("xw16", [1024, H], F16, isOutput=False)
    xo32 = nc.declare_dram_parameter("xo32", [TPC, H], F32, isOutput=False)
    wq = nc.declare_dram_parameter("wq", [H, NQ * HD], F16, isOutput=False)
    wk = nc.declare_dram_parameter("wk", [H, 2 * NKV * HD], F16, isOutput=False)
    wv = nc.declare_dram_parameter("wv", [H, NKV * HD], F16, isOutput=False)
    wo = nc.declare_dram_parameter("wo", [NQ * HD, H], F16, isOutput=False)
    wr = nc.declare_dram_parameter("wr", [H, E], F32, isOutput=False)
    wg = nc.declare_dram_parameter("wg", [H, I], F16, isOutput=False)
    wu = nc.declare_dram_parameter("wu", [H, I], F16, isOutput=False)
    wd = nc.declare_dram_parameter("wd", [I, H], F16, isOutput=False)
    cosw = nc.declare_dram_parameter("cosw", [128, 1024], F32, isOutput=False)
    sinw = nc.declare_dram_parameter("sinw", [128, 1024], F32, isOutput=False)
    m4 = nc.declare_dram_parameter("m4", [4, 128, 640], F16, isOutput=False)
    rotT = nc.declare_dram_parameter("rotT", [128, 128], F32, isOutput=False)
    id16 = nc.declare_dram_parameter("id16", [128, 128], F16, isOutput=False)
    id32 = nc.declare_dram_parameter("id32", [128, 128], F32, isOutput=False)
    u128 = nc.declare_dram_parameter("u128", [128, 128], F16, isOutput=False)
    sl32 = nc.declare_dram_parameter("sl32", [32, 32], F32, isOutput=False)
    ones_l = nc.declare_dram_parameter("ones_l", [32, 128], F16, isOutput=False)
    iota_w = nc.declare_dram_parameter("iota_w", [128, 2 * TC], F32, isOutput=False)
    esel = nc.declare_dram_parameter("esel", [128, E], F32, isOutput=False)

    out_dec = nc.declare_dram_parameter("out_dec", [TPC, H], F32, isOutput=True)
    out_rl = nc.declare_dram_parameter("out_rl", [TPC, E], F32, isOutput=True)
    if _DEBUG:
        dbg_x2 = nc.declare_dram_parameter("dbg_x2", [TPC, H], F32, isOutput=True)
        dbg_moe = nc.declare_dram_parameter("dbg_moe", [TPC, H], F32, isOutput=True)

    # ---------------- internal DRAM ----------------
    ag_in = nc.dram_tensor("ag_in", [TPC, H], F16)
    ag_hs = nc.dram_tensor("ag_hs", [T + 128, H], F16, addr_space="Shared")
    lg_in = nc.dram_tensor("lg_in", [TPC, E], F32)
    ag_lg = nc.dram_tensor("ag_lg", [T, E], F32, addr_space="Shared")
    tmds = [nc.dram_tensor(f"tmap_dram{k}", [TMAPN, 2], F32) for k in range(8)]
    accum = nc.dram_tensor("accum", [T + 128, H], F16)
    rs_out = nc.dram_tensor("rs_out", [TPC, H], F16)

    with tile.TileContext(nc) as tc:
        # persistent pool (consts + tensors alive across phases)
        pp = tc.alloc_tile_pool(name="persist", bufs=1)
        c_id16 = pp.tile([128, 128], F16)
        c_id32 = pp.tile([128, 128], F32)
        c_u128 = pp.tile([128, 128], F16)
        c_sl32 = pp.tile([32, 32], F32)
        c_onesl = pp.tile([32, 128], F16)
        c_rotT = pp.tile([128, 128], F32)
        c_esel = pp.tile([128, E], F32)
        tw = pp.tile([128, 2 * TC], F32)
        nc.sync.dma_start(out=c_id16[:], in_=id16[:, :])
        nc.sync.dma_start(out=c_id32[:], in_=id32[:, :])
        nc.sync.dma_start(out=c_u128[:], in_=u128[:, :])
        nc.sync.dma_start(out=c_sl32[:], in_=sl32[:, :])
        nc.sync.dma_start(out=c_onesl[:], in_=ones_l[:, :])
        nc.sync.dma_start(out=c_rotT[:], in_=rotT[:, :])
        nc.sync.dma_start(out=c_esel[:], in_=esel[:, :])
        nc.sync.dma_start(out=tw[:], in_=iota_w[:, :])
        x2 = pp.tile([128, 4, H], F32)          # post-attention + residual (own tokens)
        logit_sb = pp.tile([128, 4, E], F32)    # own-slice router logits

        # =========== Phase A/B/C: attention ===========
        ap = tc.alloc_tile_pool(name="attn", bufs=1)
        ap1 = tc.alloc_tile_pool(name="attn1", bufs=1)
        ap2 = tc.alloc_tile_pool(name="attn2", bufs=2)
        aps = tc.alloc_tile_pool(name="attn_ps", bufs=3, space="PSUM")
        aps2 = tc.alloc_tile_pool(name="attn_ps2", bufs=2, space="PSUM")

        c_cos = ap.tile([128, 1024], F32)
        c_sin = ap.tile([128, 1024], F32)
        c_m4 = ap.tile([128, 4, 640], F16)
        nc.sync.dma_start(out=c_cos[:], in_=cosw[:, :])
        nc.sync.dma_start(out=c_sin[:], in_=sinw[:, :])
        nc.sync.dma_start(out=c_m4[:], in_=m4.ap().rearrange("q p k -> p q k"))
        wq_sb = ap.tile([128, 8, NQ * HD], F16)
        wk_sb = ap.tile([128, 8, 2 * NKV * HD], F16)
        wv_sb = ap.tile([128, 8, NKV * HD], F16)
        wo_sb = ap.tile([128, 8, H], F16)
        nc.sync.dma_start(out=wq_sb[:], in_=wq.ap().rearrange("(hc p) n -> p hc n", p=128))
        nc.sync.dma_start(out=wk_sb[:], in_=wk.ap().rearrange("(hc p) n -> p hc n", p=128))
        nc.sync.dma_start(out=wv_sb[:], in_=wv.ap().rearrange("(hc p) n -> p hc n", p=128))
        nc.sync.dma_start(out=wo_sb[:], in_=wo.ap().rearrange("(nc p) h -> p nc h", p=128))

        # A: rmsnorm(x_win) -> hs (token-major), then transpose -> hsT
        hsT = ap.tile([128, 8, 1024], F16)      # [h%128, h//128, window tok]
        for tt in range(8):
            xt = ap2.tile([128, H], F16, tag="xw")
            nc.sync.dma_start(out=xt[:], in_=xw16[128 * tt:128 * (tt + 1), :])
            sq = ap1.tile([128, H], F32, tag="sq")
            nc.vector.tensor_tensor(out=sq[:], in0=xt[:], in1=xt[:], op=AOP.mult)
            ssum = ap2.tile([128, 1], F32, tag="ssum")
            nc.vector.tensor_reduce(out=ssum[:], in_=sq[:], axis=mybir.AxisListType.X, op=AOP.add)
            nc.vector.tensor_scalar(out=ssum[:], in0=ssum[:], scalar1=1.0 / H, op0=AOP.mult,
                                    scalar2=EPS, op1=AOP.add)
            nc.vector.reciprocal(out=ssum[:], in_=ssum[:])
            rs_ = ap2.tile([128, 1], F32, tag="rs")
            nc.scalar.activation(rs_[:], ssum[:], AF.Sqrt)
            hst = ap2.tile([128, H], F16, tag="hst")
            nc.vector.tensor_scalar_mul(hst[:], xt[:], rs_[:, :1])
            for hc in range(8):
                tp = aps2.tile([128, 128], F16, space="PSUM", tag="tp")
                nc.tensor.transpose(out=tp[:], in_=hst[:, 128 * hc:128 * (hc + 1)], identity=c_id16[:])
                nc.scalar.copy(out=hsT[:, hc, 128 * tt:128 * (tt + 1)], in_=tp[:])

        # B: QKV projections + rope
        qroT = ap.tile([128, 8, 512], F16)      # [(2 heads)*64, m, q tok]
        kroT = ap.tile([128, 4, 1024], F16)
        v16 = ap.tile([128, 8, NKV * HD], F16)  # token-major V
        for m in range(8):
            pq = aps.tile([128, 1024], F32, space="PSUM", tag="sb")
            for hc in range(8):
                nc.tensor.matmul(pq[:, 0:512], wq_sb[:, hc, 128 * m:128 * (m + 1)],
                                 hsT[:, hc, 512:1024], start=(hc == 0), stop=(hc == 7))
            qsb = ap2.tile([128, 512], F32, tag="qsb")
            nc.vector.tensor_copy(qsb[:], pq[:, 0:512])
            prot = aps.tile([128, 1024], F32, space="PSUM", tag="sb")
            nc.tensor.matmul(prot[:, 0:512], c_rotT[:], qsb[:], start=True, stop=True)
            t1 = ap2.tile([128, 512], F32, tag="t1")
            nc.vector.tensor_tensor(out=t1[:], in0=qsb[:], in1=c_cos[:, 512:1024], op=AOP.mult)
            t2 = ap2.tile([128, 512], F32, tag="t2")
            nc.vector.tensor_copy(t2[:], prot[:, 0:512])
            nc.vector.tensor_tensor(out=t2[:], in0=t2[:], in1=c_sin[:, 512:1024], op=AOP.mult)
            nc.vector.tensor_tensor(out=qroT[:, m, :], in0=t1[:], in1=t2[:], op=AOP.add)
        for m in range(4):
            pk = aps.tile([128, 1024], F32, space="PSUM", tag="sb")
            for half in range(2):
                for hc in range(8):
                    nc.tensor.matmul(pk[:, 512 * half:512 * (half + 1)],
                                     wk_sb[:, hc, 128 * m:128 * (m + 1)],
                                     hsT[:, hc, 512 * half:512 * (half + 1)],
                                     start=(hc == 0), stop=(hc == 7))
            ksb = ap1.tile([128, 1024], F32, tag="ksb")
            nc.vector.tensor_copy(ksb[:], pk[:])
            prot = aps.tile([128, 1024], F32, space="PSUM", tag="sb")
            nc.tensor.matmul(prot[:, 0:512], c_rotT[:], ksb[:, 0:512], start=True, stop=True)
            nc.tensor.matmul(prot[:, 512:1024], c_rotT[:], ksb[:, 512:1024], start=True, stop=True)
            t1 = ap1.tile([128, 1024], F32, tag="t1k")
            nc.vector.tensor_tensor(out=t1[:], in0=ksb[:], in1=c_cos[:], op=AOP.mult)
            t2 = ap1.tile([128, 1024], F32, tag="t2k")
            nc.vector.tensor_copy(t2[:], prot[:])
            nc.vector.tensor_tensor(out=t2[:], in0=t2[:], in1=c_sin[:], op=AOP.mult)
            nc.vector.tensor_tensor(out=kroT[:, m, :], in0=t1[:], in1=t2[:], op=AOP.add)
        for tt in range(8):
            pv = aps.tile([128, NKV * HD], F32, space="PSUM", tag="sb")
            for hc in range(8):
                nc.tensor.matmul(pv[:], hsT[:, hc, 128 * tt:128 * (tt + 1)],
                                 wv_sb[:, hc, :], start=(hc == 0), stop=(hc == 7))
            nc.vector.tensor_copy(v16[:, tt, :], pv[:])

        # C: per q-tile attention
        wr_sb = ap.tile([128, 8, E], F32)
        nc.sync.dma_start(out=wr_sb[:], in_=wr.ap().rearrange("(hc p) e -> p hc e", p=128))
        for qt in range(4):
            acat = ap.tile([128, 8, 128], F16, tag="acat")
            for kh in range(NKV):
                for ql in range(4):
                    qh = 4 * kh + ql
                    ps = aps.tile([128, 640], F32, space="PSUM", tag="sb")
                    par = 64 * (qh % 2)
                    lq = qroT[par:par + 64, qh // 2, 128 * qt:128 * (qt + 1)]
                    rk = kroT[par:par + 64, kh, :]
                    nc.tensor.matmul(ps[:, 0:512], lq, rk[:, 128 * qt:128 * qt + 512],
                                     start=True, stop=True)
                    nc.tensor.matmul(ps[:, 512:640], lq, rk[:, 128 * qt + 512:128 * qt + 640],
                                     start=True, stop=True)
                    nc.vector.tensor_tensor(out=ps[:], in0=ps[:], in1=c_m4[:, qt, :], op=AOP.add)
                    mx = ap2.tile([128, 1], F32, tag="mx")
                    nc.vector.tensor_reduce(out=mx[:], in_=ps[:], axis=mybir.AxisListType.X, op=AOP.max)
                    nc.vector.tensor_scalar_mul(mx[:], mx[:], -1.0)
                    pr = ap2.tile([128, 640], F16, tag="pr")
                    nc.scalar.activation(pr[:], ps[:], AF.Exp, bias=mx[:, :1])
                    se = ap2.tile([128, 1], F32, tag="se")
                    nc.vector.tensor_reduce(out=se[:], in_=pr[:], axis=mybir.AxisListType.X, op=AOP.add)
                    nc.vector.reciprocal(out=se[:], in_=se[:])
                    nc.vector.tensor_scalar_mul(pr[:], pr[:], se[:, :1])
                    prT = ap2.tile([128, 5, 128], F16, tag="prT")
                    for kc in range(5):
                        tp = aps2.tile([128, 128], F16, space="PSUM", tag="tp")
                        nc.tensor.transpose(out=tp[:], in_=pr[:, 128 * kc:128 * (kc + 1)],
                                            identity=c_id16[:])
                        nc.scalar.copy(out=prT[:, kc, :], in_=tp[:])
                    pa = aps2.tile([64, 128], F32, space="PSUM", tag="tp")
                    for kc in range(5):
                        nc.tensor.matmul(pa[:], v16[:, qt + kc, 64 * kh:64 * kh + 64],
                                         prT[:, kc, :], start=(kc == 0), stop=(kc == 4))
                    patmp = ap2.tile([64, 128], F16, tag="patmp")
                    nc.scalar.copy(out=patmp[:], in_=pa[:])
                    nc.sync.dma_start(out=acat[64 * (qh % 2):64 * (qh % 2) + 64, qh // 2, :],
                                      in_=patmp[:])
            po = aps.tile([128, 1024], F32, space="PSUM", tag="sb")
            for half in range(2):
                for b in range(8):
                    nc.tensor.matmul(po[:, 512 * half:512 * (half + 1)], acat[:, b, :],
                                     wo_sb[:, b, 512 * half:512 * (half + 1)],
                                     start=(b == 0), stop=(b == 7))
            xo_t = ap1.tile([128, H], F32, tag="xo")
            nc.sync.dma_start(out=xo_t[:], in_=xo32[128 * qt:128 * (qt + 1), :])
            nc.vector.tensor_tensor(out=x2[:, qt, :], in0=po[:], in1=xo_t[:], op=AOP.add)

            # norm2 + hs2 (fp16, to AG) + hs2T (f32, router)
            sq = ap1.tile([128, H], F32, tag="sq2")
            nc.vector.tensor_tensor(out=sq[:], in0=x2[:, qt, :], in1=x2[:, qt, :], op=AOP.mult)
            ssum = ap2.tile([128, 1], F32, tag="ssum2")
            nc.vector.tensor_reduce(out=ssum[:], in_=sq[:], axis=mybir.AxisListType.X, op=AOP.add)
            nc.vector.tensor_scalar(out=ssum[:], in0=ssum[:], scalar1=1.0 / H, op0=AOP.mult,
                                    scalar2=EPS, op1=AOP.add)
            nc.vector.reciprocal(out=ssum[:], in_=ssum[:])
            rs_ = ap2.tile([128, 1], F32, tag="rs2")
            nc.scalar.activation(rs_[:], ssum[:], AF.Sqrt)
            hs2_32 = ap1.tile([128, H], F32, tag="hs2_32")
            nc.vector.tensor_scalar_mul(hs2_32[:], x2[:, qt, :], rs_[:, :1])
            hs2_16 = ap2.tile([128, H], F16, tag="hs2_16")
            nc.vector.tensor_copy(hs2_16[:], hs2_32[:])
            nc.sync.dma_start(out=ag_in[128 * qt:128 * (qt + 1), :], in_=hs2_16[:])
            # router: logits = hs2 @ wr  (f32; lhsT = hs2T chunks)
            plg = aps2.tile([128, E], F32, space="PSUM", tag="tp")
            for hc in range(8):
                tp32 = aps2.tile([128, 128], F32, space="PSUM", tag="tp")
                nc.tensor.transpose(out=tp32[:], in_=hs2_32[:, 128 * hc:128 * (hc + 1)],
                                    identity=c_id32[:])
                h2T = ap2.tile([128, 128], F32, tag="h2T")
                nc.vector.tensor_copy(h2T[:], tp32[:])
                nc.tensor.matmul(plg[:], h2T[:], wr_sb[:, hc, :], start=(hc == 0), stop=(hc == 7))
            nc.vector.tensor_copy(logit_sb[:, qt, :], plg[:])
            nc.sync.dma_start(out=lg_in[128 * qt:128 * (qt + 1), :], in_=logit_sb[:, qt, :])
            nc.sync.dma_start(out=out_rl[128 * qt:128 * (qt + 1), :], in_=logit_sb[:, qt, :])
            if _DEBUG:
                nc.sync.dma_start(out=dbg_x2[128 * qt:128 * (qt + 1), :], in_=x2[:, qt, :])

        for pool in (aps2, aps, ap2, ap1, ap):
            pool.release()

        # =========== AG collectives ===========
        cp = tc.alloc_tile_pool(name="coll", bufs=1)
        zt = cp.tile([128, H], F16)
        nc.vector.memset(zt[:], 0.0)
        nc.sync.dma_start(out=ag_hs[T:T + 128, :], in_=zt[:])
        nc.gpsimd.collective_compute("AllGather", AOP.bypass, replica_groups=RG,
                                     ins=[ag_in[:, :]], outs=[ag_hs[0:T, :]])
        nc.gpsimd.collective_compute("AllGather", AOP.bypass, replica_groups=RG,
                                     ins=[lg_in[:, :]], outs=[ag_lg[:, :]])

        # =========== Phase E: gating over all T tokens ===========
        gp = tc.alloc_tile_pool(name="gate", bufs=1)
        gp2 = tc.alloc_tile_pool(name="gate2", bufs=3)
        gps = tc.alloc_tile_pool(name="gate_ps", bufs=2, space="PSUM")
        mask32 = gp.tile([128, TC], F32)
        lt_all = gp.tile([128, TC, E], F32)
        nc.sync.dma_start(out=lt_all[:], in_=ag_lg.ap().rearrange("(c p) e -> p c e", p=128))
        m1 = gp.tile([128, TC], F32)
        nc.vector.tensor_reduce(out=m1[:], in_=lt_all[:], axis=mybir.AxisListType.X, op=AOP.max)
        m1b = m1[:].to_broadcast([128, TC, E])
        iseq = gp.tile([128, TC, E], F32)
        nc.vector.tensor_tensor(out=iseq[:], in0=lt_all[:], in1=m1b, op=AOP.is_equal)
        lt2 = gp.tile([128, TC, E], F32)
        nc.vector.scalar_tensor_tensor(out=lt2[:], in0=iseq[:], scalar=-1e9, in1=lt_all[:],
                                       op0=AOP.mult, op1=AOP.add)
        m2 = gp.tile([128, TC], F32)
        nc.vector.tensor_reduce(out=m2[:], in_=lt2[:], axis=mybir.AxisListType.X, op=AOP.max)
        eselb = c_esel[:].rearrange("p e -> p () e").to_broadcast([128, TC, E])
        lesel = gp.tile([128, TC, E], F32)
        nc.vector.tensor_tensor(out=lesel[:], in0=lt_all[:], in1=eselb, op=AOP.mult)
        le = gp.tile([128, TC], F32)
        nc.vector.tensor_reduce(out=le[:], in_=lesel[:], axis=mybir.AxisListType.X, op=AOP.add)
        nc.vector.tensor_tensor(out=mask32[:], in0=le[:], in1=m2[:], op=AOP.is_ge)
        d21 = gp.tile([128, TC], F32)
        nc.vector.tensor_tensor(out=d21[:], in0=m2[:], in1=m1[:], op=AOP.subtract)
        e2 = gp.tile([128, TC], F32)
        nc.scalar.activation(e2[:], d21[:], AF.Exp)
        nc.vector.tensor_scalar_add(e2[:], e2[:], 1.0)
        nc.vector.reciprocal(out=e2[:], in_=e2[:])
        dle = gp.tile([128, TC], F32)
        nc.vector.tensor_tensor(out=dle[:], in0=le[:], in1=m1[:], op=AOP.subtract)
        ew = gp.tile([128, TC], F32)
        nc.scalar.activation(ew[:], dle[:], AF.Exp)
        nc.vector.tensor_tensor(out=ew[:], in0=ew[:], in1=e2[:], op=AOP.mult)
        nc.vector.tensor_tensor(out=ew[:], in0=ew[:], in1=mask32[:], op=AOP.mult)
        nc.vector.tensor_copy(tw[:, 1:2 * TC:2], ew[:])

        # =========== Phase F: dispatch ===========
        mask16 = gp.tile([128, TC], F16)
        nc.vector.tensor_copy(mask16[:], mask32[:])
        colcum = gps.tile([128, TC], F32, space="PSUM", tag="colcum")
        nc.tensor.matmul(colcum[:], c_u128[:], mask16[:], start=True, stop=True)
        ones128 = gp.tile([128, 1], F16)
        nc.vector.memset(ones128[:], 1.0)
        counts_p = gps.tile([32, 1], F32, space="PSUM", tag="counts")
        nc.tensor.matmul(counts_p[:], mask16[:], ones128[:], start=True, stop=True)
        counts = gp.tile([32, 1], F32)
        nc.vector.tensor_copy(counts[:], counts_p[:])
        slc = gp.tile([32, 32], F16)
        nc.vector.tensor_scalar_mul(slc[:], c_sl32[:], counts[:, :1])
        carry = gps.tile([128, TC], F32, space="PSUM", tag="carry")
        nc.tensor.matmul(carry[:], c_onesl[:], slc[:, :TC], start=True, stop=True)
        pos = gp.tile([128, TC], F32)
        nc.vector.tensor_copy(pos[:], colcum[:])
        nc.vector.tensor_tensor(out=pos[:], in0=pos[:], in1=carry[:], op=AOP.add)
        nc.vector.tensor_tensor(out=pos[:], in0=pos[:], in1=mask32[:], op=AOP.subtract)
        nc.vector.tensor_scalar_add(pos[:], pos[:], -float(TRASH))
        nc.vector.tensor_tensor(out=pos[:], in0=pos[:], in1=mask32[:], op=AOP.mult)
        nc.vector.tensor_scalar_add(pos[:], pos[:], float(TRASH))
        nc.vector.tensor_scalar_min(pos[:], pos[:], float(TMAPN - 1))
        pos_i = gp.tile([128, TC], I32)
        nc.vector.tensor_copy(pos_i[:], pos[:])

        pre = gp.tile([128, 2 * TMAPN // 128], F32)
        nc.vector.memset(pre[:], 0.0)
        for k in range(8):
            nc.sync.dma_start(out=tmds[k].ap().rearrange("(f p) x -> p f x", p=128),
                              in_=pre[:].rearrange("p (f x) -> p f x", x=2))
        for ci in range(TC):
            nc.gpsimd.indirect_dma_start(
                out=tmds[ci % 8][:, :],
                out_offset=bass.IndirectOffsetOnAxis(ap=pos_i[:, ci:ci + 1], axis=0),
                in_=tw[:, 2 * ci:2 * ci + 2], in_offset=None)

        # merge stripes: each slot written by exactly one stripe (ids are +1, zeros elsewhere)
        tmap_f = gp.tile([128, CF], F32)
        wz = gp.tile([128, CF], F32)
        for k in range(8):
            tpart = gp2.tile([128, CF], F32, tag="tpart", name="tpart")
            nc.sync.dma_start(out=tpart[:],
                              in_=tmds[k].ap()[:C, 0:1].rearrange("(m p) x -> p (m x)", p=128))
            wpart = gp2.tile([128, CF], F32, tag="wpart", name="wpart")
            nc.sync.dma_start(out=wpart[:],
                              in_=tmds[k].ap()[:C, 1:2].rearrange("(m p) x -> p (m x)", p=128))
            if k == 0:
                nc.vector.tensor_copy(tmap_f[:], tpart[:])
                nc.vector.tensor_copy(wz[:], wpart[:])
            else:
                nc.vector.tensor_tensor(out=tmap_f[:], in0=tmap_f[:], in1=tpart[:], op=AOP.add)
                nc.vector.tensor_tensor(out=wz[:], in0=wz[:], in1=wpart[:], op=AOP.add)
        # ids were scattered as id+1; empty slots (0) -> dummy token T
        zeroq = gp.tile([128, CF], F32)
        nc.vector.tensor_scalar(out=zeroq[:], in0=tmap_f[:], scalar1=0.0, op0=AOP.is_equal, scalar2=0.0, op1=AOP.add)
        nc.vector.tensor_scalar_add(tmap_f[:], tmap_f[:], -1.0)
        nc.vector.tensor_scalar_mul(zeroq[:], zeroq[:], float(T + 1))
        nc.vector.tensor_tensor(out=tmap_f[:], in0=tmap_f[:], in1=zeroq[:], op=AOP.add)
        tmap_i = gp.tile([128, CF], I32)
        nc.vector.tensor_copy(tmap_i[:], tmap_f[:])

        gath = gp.tile([128, CF, H], F16)
        for m in range(CF):
            nc.gpsimd.indirect_dma_start(
                out=gath[:, m, :], out_offset=None, in_=ag_hs[:, :],
                in_offset=bass.IndirectOffsetOnAxis(ap=tmap_i[:, m:m + 1], axis=0))
        hsTc = gp.tile([128, 8, C], F16)
        for m in range(CF):
            for hc in range(8):
                tp = gps.tile([128, 128], F16, space="PSUM", tag="tpg")
                nc.tensor.transpose(out=tp[:], in_=gath[:, m, 128 * hc:128 * (hc + 1)],
                                    identity=c_id16[:])
                nc.scalar.copy(out=hsTc[:, hc, 128 * m:128 * (m + 1)], in_=tp[:])
        gps.release()

        # zero accum rows (all T+128)
        za = gp.tile([128, H], F16)
        nc.vector.memset(za[:], 0.0)
        for i_ in range(T // 128 + 1):
            nc.sync.dma_start(out=accum[128 * i_:128 * (i_ + 1), :], in_=za[:])

        # =========== Phase G: expert FFN over C slots ===========
        fp_ = tc.alloc_tile_pool(name="ffn", bufs=1)
        fp2 = tc.alloc_tile_pool(name="ffn2", bufs=3)
        fps1 = tc.alloc_tile_pool(name="ffn_ps1", bufs=1, space="PSUM")
        fps2 = tc.alloc_tile_pool(name="ffn_ps2", bufs=1, space="PSUM")
        act_blk = fp_.tile([128, IT, TB], F16, tag="act")
        for blk in range(NB):
            cols = slice(TB * blk, TB * (blk + 1))
            for it in range(IT):
                wgS = fp2.tile([128, 8, 128], F16, tag="wgS")
                nc.sync.dma_start(out=wgS[:],
                                  in_=wg.ap()[:, 128 * it:128 * (it + 1)]
                                  .rearrange("(hc p) i -> p hc i", p=128))
                wuS = fp2.tile([128, 8, 128], F16, tag="wuS")
                nc.sync.dma_start(out=wuS[:],
                                  in_=wu.ap()[:, 128 * it:128 * (it + 1)]
                                  .rearrange("(hc p) i -> p hc i", p=128))
                pg = fps1.tile([128, TB], F32, space="PSUM", tag="pg")
                pu = fps1.tile([128, TB], F32, space="PSUM", tag="pu")
                for hc in range(8):
                    nc.tensor.matmul(pg[:], wgS[:, hc, :],
                                     hsTc[:, hc, cols], start=(hc == 0), stop=(hc == 7))
                for hc in range(8):
                    nc.tensor.matmul(pu[:], wuS[:, hc, :],
                                     hsTc[:, hc, cols], start=(hc == 0), stop=(hc == 7))
                sg = fp2.tile([128, TB], F16, tag="sg")
                nc.scalar.activation(sg[:], pg[:], AF.Silu)
                nc.vector.tensor_tensor(out=act_blk[:, it, :], in0=sg[:], in1=pu[:], op=AOP.mult)
            pys = [fps2.tile([128, H], F32, space="PSUM", tag=f"py{ms}", name=f"py{ms}") for ms in range(3)]
            for it in range(IT):
                wdS = fp2.tile([128, H], F16, tag="wdS")
                nc.sync.dma_start(out=wdS[:], in_=wd[128 * it:128 * (it + 1), :])
                for ms in range(3):
                    for half in range(2):
                        nc.tensor.matmul(pys[ms][:, 512 * half:512 * (half + 1)],
                                         act_blk[:, it, 128 * ms:128 * (ms + 1)],
                                         wdS[:, 512 * half:512 * (half + 1)],
                                         start=(it == 0), stop=(it == IT - 1))
            for ms in range(3):
                sidx = 3 * blk + ms
                y16 = fp2.tile([128, H], F16, tag="y16")
                nc.vector.tensor_scalar_mul(y16[:], pys[ms][:], wz[:, sidx:sidx + 1])
                nc.gpsimd.indirect_dma_start(
                    out=accum[:, :],
                    out_offset=bass.IndirectOffsetOnAxis(ap=tmap_i[:, sidx:sidx + 1], axis=0),
                    in_=y16[:], in_offset=None)

        # =========== Phase H: ReduceScatter + final residual ===========
        nc.gpsimd.collective_compute("ReduceScatter", AOP.add, replica_groups=RG,
                                     ins=[accum[0:T, :]], outs=[rs_out[:, :]])
        for qt in range(4):
            rt = fp2.tile([128, H], F16, tag="rt")
            nc.sync.dma_start(out=rt[:], in_=rs_out[128 * qt:128 * (qt + 1), :])
            if _DEBUG:
                moe32 = fp2.tile([128, H], F32, tag="moe32")
                nc.vector.tensor_copy(moe32[:], rt[:])
                nc.sync.dma_start(out=dbg_moe[128 * qt:128 * (qt + 1), :], in_=moe32[:])
            fin = fp2.tile([128, H], F32, tag="fin")
            nc.vector.tensor_tensor(out=fin[:], in0=rt[:], in1=x2[:, qt, :], op=AOP.add)
            nc.sync.dma_start(out=out_dec[128 * qt:128 * (qt + 1), :], in_=fin[:])

        for pool in (fps2, fps1, fp2, fp_, gp2, gp, cp, pp):
            pool.release()

    nc.compile()
    return nc


def _host_inputs(inputs):
    x = np.asarray(inputs["decoder_sequence"], np.float32)        # [B, S, H]
    wq = np.asarray(inputs["wq"], np.float32).reshape(H, NQ * HD) / 8.0
    wk_ = np.asarray(inputs["wk"], np.float32).reshape(H, NKV, HD)
    wk = np.zeros((H, NKV, 2, HD), np.float32)
    wk[:, :, 0] = wk_
    wk[:, :, 1] = wk_
    wk = wk.reshape(H, 2 * NKV * HD)
    wv = np.asarray(inputs["wv"], np.float32).reshape(H, NKV * HD)
    wo = np.asarray(inputs["wo"], np.float32).reshape(NQ * HD, H)
    wr = np.asarray(inputs["w_router"], np.float32)
    wg = np.asarray(inputs["w_gate"], np.float32)
    wu_ = np.asarray(inputs["w_up"], np.float32)
    wd_ = np.asarray(inputs["w_down"], np.float32)

    f16 = lambda a: a.astype(np.float16)

    # constants
    id128 = np.eye(128, dtype=np.float32)
    u128 = np.triu(np.ones((128, 128), np.float16))
    sl32 = np.triu(np.ones((32, 32), np.float32), 1)
    ones_l = np.ones((32, 128), np.float16)
    iota_w = np.zeros((128, 2 * TC), np.float32)
    for ci in range(TC):
        iota_w[:, 2 * ci] = ci * 128 + np.arange(128) + 1
    M = np.zeros((64, 64), np.float32)
    for i_ in range(32):
        M[2 * i_, 32 + i_] = -1.0
        M[2 * i_ + 1, i_] = 1.0
    rot = np.zeros((128, 128), np.float32)
    rot[:64, :64] = M
    rot[64:, 64:] = M
    rotT = rot.T.copy()

    inv = 1.0 / (10000.0 ** (np.arange(0, HD, 2, dtype=np.float64) / HD))  # [32]

    in_maps = []
    for c in range(CORES):
        b, s0 = c // 4, (c % 4) * TPC
        own = x[b, s0:s0 + TPC]                                   # [512, H]
        prev = x[b, s0 - WIN:s0] if s0 >= WIN else np.zeros((WIN, H), np.float32)
        xw = np.concatenate([prev, own], 0)                       # [1024, H]
        posw = np.arange(s0 - WIN, s0 + TPC)
        posc = np.maximum(posw, 0).astype(np.float64)
        emb = posc[None, :] * inv[:, None]                        # [32, 1024]
        cos64 = np.repeat(np.cos(emb), 2, axis=0).astype(np.float32)   # [64, 1024]
        sin64 = np.repeat(np.sin(emb), 2, axis=0).astype(np.float32)
        cosw = np.concatenate([cos64, cos64], 0)                  # [128, 1024]
        sinw = np.concatenate([sin64, sin64], 0)
        m4 = np.full((4, 128, 640), NEG, np.float16)
        ii = np.arange(128)[:, None]
        jj = np.arange(640)[None, :]
        for qt in range(4):
            kg = s0 - WIN + 128 * qt + jj
            valid = (jj > ii) & (jj <= ii + WIN) & (kg >= 0)
            m4[qt][valid.nonzero()] = 0.0
        esel = np.zeros((128, E), np.float32)
        esel[:, c] = 1.0
        in_maps.append({
            "xw16": f16(xw), "xo32": own,
            "wq": f16(wq), "wk": f16(wk), "wv": f16(wv), "wo": f16(wo),
            "wr": wr, "wg": f16(wg[c]), "wu": f16(wu_[c]), "wd": f16(wd_[c]),
            "cosw": cosw, "sinw": sinw, "m4": m4, "rotT": rotT,
            "id16": np.eye(128, dtype=np.float16), "id32": id128,
            "u128": u128, "sl32": sl32, "ones_l": ones_l,
            "iota_w": iota_w, "esel": esel,
        })
    return in_maps


def kernel(**inputs):
    global _nc_cache
    if _nc_cache is None:
        _nc_cache = build_kernel()
    in_maps = _host_inputs(inputs)
    res = run_bass_kernel_spmd(_nc_cache, in_maps, core_ids=list(range(CORES)),
                               trace=bool(int(os.environ.get("BASS_MOE_TRACE", "0"))))
    kernel.last_results = res
    dec = np.concatenate([res.results[c]["out_dec"] for c in range(CORES)], 0)
    rl = np.concatenate([res.results[c]["out_rl"] for c in range(CORES)], 0)
    return dec.reshape(B, S, H).astype(np.float32), rl.astype(np.float32)
